# revision 1
# baseline (speedup 1.0000x reference)
"""Trainium2 distributed kernel for ABRLovaszCELoss (8 NeuronCores).

Strategy (v3)
-------------
Data-parallel over (batch, row-half): core i handles batch b=i//2, fine rows
[192*(i%2), 192*(i%2)+192) of the 384x384 target grid (73728 pixels/core).

Per core, fully on-device:
- bilinear align_corners upsample 96->384 of all 19 logit channels
  (order head1:3, head2:2, head0:7, dsn:7) as two PE matmuls per channel;
  transposed dataflow avoids on-chip transposes.  Pixel layout
  [128 part = X%128, 576 free = 192*(X//128) + fy].
- stage-1 PSUM copies on vector; stage-2 exp(z) fused into the PSUM->SBUF
  copy on scalar; CE's sum(z*[tgt==c]) accumulated by vector DIRECTLY from
  stage-2 PSUM (raw z never hits SBUF).
- softmax reciprocal as exp(-ln(S)) on the scalar engine; the head0 Ln pass
  also accumulates CE's sum(ln S) for free.
- Lovasz-Softmax per shard via exact relu tail-integrals instead of a sort:
  with x = [tgt==c] - p_c (stored bf16),
     TF_j = sum relu(x - t_j),  TB_j = sum relu(-x - t_j)
  exact per-segment integrals by differencing, and
     L_c ~= sum_j (IF_j + IB_j) / (n_c + K*IB_j).
  Fused compare+accumulate passes split across scalar and vector engines.
  Per-shard averaging error ~1e-6; quadrature+bf16 error ~2e-4.
- per-class finale on one partition (step-interleaved across the 12 classes
  to dodge small-tile RAW hazards), single-scalar AllReduce at the end.
"""

import numpy as np

import concourse.bass as bass
import concourse.mybir as mybir
from concourse.bass_utils import run_bass_kernel_spmd

F32 = mybir.dt.float32
BF16 = mybir.dt.bfloat16
AF = mybir.ActivationFunctionType
OP = mybir.AluOpType

NCH = 19
# channel order: head1 (3), head2 (2), head0 (7), dsn (7)
H1 = list(range(0, 3))
H2 = list(range(3, 5))
H0 = list(range(5, 12))
HD = list(range(12, 19))
K0 = 4
K12 = 8
P_GLOBAL = 4 * 384 * 384

# acc_sb columns: class i base=40*i: TB +0..K-1 (pad +K), zfg0 chunks +8..10,
# zfgd chunks +12..14 (head0 bases only), TF +20..20+K-1 (pad), n_c +38,
# contrib +39.  CE: 480 lnS0, 488 lnSd.
ACC_W = 512
COL_LNS0 = 480
COL_LNSD = 488

CLASSES = (
    [("x0", c, K0, 40 * c) for c in range(7)]
    + [("x1", c, K12, 40 * (7 + c)) for c in range(3)]
    + [("x2", c, K12, 40 * (10 + c)) for c in range(2)]
)

DEBUG = False


def build_kernel():
    nc = bass.Bass()

    thr_vals = sorted(
        {-float(j) / K0 for j in range(1, K0)} | {-float(j) / K12 for j in range(1, K12)}
    )
    for i, val in enumerate(thr_vals):
        t = nc.alloc_sbuf_tensor(f"const-thr-{i}", [128, 1], F32)
        nc.gpsimd.memset(t.ap(), val)
        nc.const_aps.aps[(F32, val)] = t.ap()
    nc.all_engine_barrier()

    p_preds = nc.declare_dram_parameter("preds_all", [49, NCH * 96], F32, isOutput=False)
    p_uyt = nc.declare_dram_parameter("uyt", [49, 192], F32, isOutput=False)
    p_ux = nc.declare_dram_parameter("ux", [96, 384], F32, isOutput=False)
    p_tgt = nc.declare_dram_parameter("tgts", [128, 3 * 576], F32, isOutput=False)
    p_wv = nc.declare_dram_parameter("wvec", [1, ACC_W], F32, isOutput=False)
    p_out = nc.declare_dram_parameter("out", [1, 128], F32, isOutput=True)

    dbg = {}
    if DEBUG:
        for nm, shp in [("d_acc", [128, ACC_W]), ("d_rst", [1, ACC_W])]:
            dbg[nm] = nc.declare_dram_parameter(nm, shp, F32, isOutput=True)

    cc_in = nc.dram_tensor("cc_in", [1, 128], F32)
    cc_out = nc.dram_tensor("cc_out", [1, 128], F32, addr_space="Shared")
    core_ids = list(range(8))

    from contextlib import ExitStack
    with ExitStack() as es:
        def sb(name, shape, dtype=F32):
            return es.enter_context(nc.sbuf_tensor(name, shape, dtype))

        preds_sb = sb("preds_sb", [49, NCH * 96])
        uyt_sb = sb("uyt_sb", [49, 192])
        ux_sb = sb("ux_sb", [96, 384])
        tf_sb = sb("tf_sb", [128, 3 * 576])
        wv_sb = sb("wv_sb", [1, ACC_W])
        t1_sb = sb("t1_sb", [96, NCH * 192])
        e0_sb = sb("e0_sb", [128, 7 * 576])   # exp tiles (fp32) -> become p
        ed_sb = sb("ed_sb", [128, 7 * 576])
        e1_sb = sb("e1_sb", [128, 3 * 576])
        e2_sb = sb("e2_sb", [128, 2 * 576])
        xb0_sb = sb("xb0_sb", [128, 7 * 576], BF16)  # x = fg - p (bf16)
        xb1_sb = sb("xb1_sb", [128, 3 * 576], BF16)
        xb2_sb = sb("xb2_sb", [128, 2 * 576], BF16)
        s_sb = sb("s_sb", [128, 4 * 576])     # S1, S2, S0, Sd
        r_sb = sb("r_sb", [128, 3 * 576])     # R1, R2, R0
        ln_sb = sb("ln_sb", [128, 576])
        onesw_sb = sb("onesw_sb", [128, 576])
        zerob_sb = sb("zerob_sb", [128, 576], BF16)
        onescol_sb = sb("onescol_sb", [128, 1])
        junk_v = sb("junk_v", [128, 576])
        junk_vb = sb("junk_vb", [128, 576], BF16)
        junk_s = sb("junk_s", [128, 576])
        acc_sb = sb("acc_sb", [128, ACC_W])
        rst_sb = sb("rst_sb", [1, ACC_W])
        tmpA_sb = sb("tmpA_sb", [1, 256])
        tmpB_sb = sb("tmpB_sb", [1, 256])
        tmpC_sb = sb("tmpC_sb", [1, 256])
        tmpD_sb = sb("tmpD_sb", [1, 256])
        tmpE_sb = sb("tmpE_sb", [1, 256])
        tmpw_sb = sb("tmpw_sb", [1, ACC_W])
        partial_sb = sb("partial_sb", [1, 128])

        ps1 = [es.enter_context(nc.psum_tensor(f"ps1{i}", [96, 192], F32)) for i in range(2)]
        ps2 = [es.enter_context(nc.psum_tensor(f"ps2{i}", [128, 192], F32)) for i in range(5)]
        psR = es.enter_context(nc.psum_tensor("psR", [1, ACC_W], F32))

        dmaP = es.enter_context(nc.semaphore("dmaP"))
        dmaU = es.enter_context(nc.semaphore("dmaU"))
        dmaX = es.enter_context(nc.semaphore("dmaX"))
        dmaT = es.enter_context(nc.semaphore("dmaT"))
        dmaW = es.enter_context(nc.semaphore("dmaW"))
        g_sem = es.enter_context(nc.semaphore("g_sem"))
        mm_sem = es.enter_context(nc.semaphore("mm_sem"))
        cp1_sem = es.enter_context(nc.semaphore("cp1_sem"))
        cp_sem = es.enter_context(nc.semaphore("cp_sem"))
        cpz_sem = es.enter_context(nc.semaphore("cpz_sem"))
        rs_sem = es.enter_context(nc.semaphore("rs_sem"))
        vx_sem = es.enter_context(nc.semaphore("vx_sem"))
        hist_sem = es.enter_context(nc.semaphore("hist_sem"))
        fin_sem = es.enter_context(nc.semaphore("fin_sem"))
        cdma_sem = es.enter_context(nc.semaphore("cdma_sem"))
        cc_sem = es.enter_context(nc.semaphore("cc_sem"))

        def thr(K):
            return [float(j) / K for j in range(K)]

        # vx milestones: 1=S1 2=x1 3=S2 4=x2 5=S0 6=x0 7=Sd
        def zcol(c, k):
            if c in H0:
                return 40 * (c - H0[0]) + 8 + k
            return 40 * (c - HD[0]) + 12 + k

        with nc.Block() as block:

            @block.sync
            def _(sync):
                sync.dma_start(out=preds_sb[:, :], in_=p_preds[:, :]).then_inc(dmaP, 16)
                sync.dma_start(out=uyt_sb[:, :], in_=p_uyt[:, :]).then_inc(dmaU, 16)
                sync.dma_start(out=ux_sb[:, :], in_=p_ux[:, :]).then_inc(dmaX, 16)

            @block.gpsimd
            def _(gpsimd):
                gpsimd.memset(acc_sb[:, :], 0.0)
                gpsimd.memset(onesw_sb[:, :], 1.0)
                gpsimd.memset(zerob_sb[:, :], 0.0)
                gpsimd.memset(onescol_sb[:, :], 1.0)
                gpsimd.memset(partial_sb[:, :], 0.0).then_inc(g_sem)
                # collective tail
                gpsimd.wait_ge(fin_sem, 1)
                gpsimd.dma_start(out=cc_in[:, :], in_=partial_sb[0:1, :]).then_inc(cdma_sem, 16)
                gpsimd.wait_ge(cdma_sem, 16)
                gpsimd.collective_compute(
                    "AllReduce", OP.add, replica_groups=[core_ids],
                    ins=[cc_in[:, :]], outs=[cc_out[:, :]],
                ).then_inc(cc_sem)
                gpsimd.wait_ge(cc_sem, 1)
                gpsimd.dma_start(out=p_out[:, :], in_=cc_out[:, :]).then_inc(cdma_sem, 16)
                gpsimd.wait_ge(cdma_sem, 32)
                if DEBUG:
                    n = 32
                    for name, t in [("d_acc", acc_sb), ("d_rst", rst_sb)]:
                        gpsimd.dma_start(out=dbg[name][:, :], in_=t[:, :]).then_inc(cdma_sem, 16)
                        n += 16
                        gpsimd.wait_ge(cdma_sem, n)

            @block.tensor
            def _(tensor):
                tensor.wait_ge(dmaP, 16)
                tensor.wait_ge(dmaU, 16)
                for c in range(NCH):
                    if c >= 2:
                        tensor.wait_ge(cp1_sem, c - 1)
                    tensor.matmul(
                        ps1[c % 2][0:96, 0:192],
                        preds_sb[0:49, 96 * c : 96 * (c + 1)],
                        uyt_sb[0:49, 0:192],
                        start=True, stop=True,
                    ).then_inc(mm_sem)
                tensor.wait_ge(dmaX, 16)
                for c in range(NCH):
                    for k in range(3):
                        idx = 3 * c + k
                        if k == 0:
                            tensor.wait_ge(cp1_sem, c + 1)
                        if idx >= 5:
                            old = idx - 5
                            tensor.wait_ge(cp_sem, old + 1)
                            if old >= 15:
                                tensor.wait_ge(cpz_sem, old - 14)
                        tensor.matmul(
                            ps2[idx % 5][0:128, 0:192],
                            ux_sb[0:96, 128 * k : 128 * (k + 1)],
                            t1_sb[0:96, 192 * c : 192 * (c + 1)],
                            start=True, stop=True,
                        ).then_inc(mm_sem)
                tensor.wait_ge(hist_sem, 2)
                tensor.matmul(
                    psR[0:1, 0:ACC_W],
                    onescol_sb[0:128, 0:1],
                    acc_sb[0:128, 0:ACC_W],
                    start=True, stop=True,
                ).then_inc(mm_sem)

            @block.scalar
            def _(scalar):
                scalar.dma_start(out=tf_sb[:, :], in_=p_tgt[:, :]).then_inc(dmaT, 16)
                scalar.dma_start(out=wv_sb[:, :], in_=p_wv[:, :]).then_inc(dmaW, 16)

                def exp_copy(c, k):
                    idx = 3 * c + k
                    scalar.wait_ge(mm_sem, 19 + idx + 1)
                    src = ps2[idx % 5][0:128, 0:192]
                    if c in H1:
                        dst_t, ci = e1_sb, c - H1[0]
                    elif c in H2:
                        dst_t, ci = e2_sb, c - H2[0]
                    elif c in H0:
                        dst_t, ci = e0_sb, c - H0[0]
                    else:
                        dst_t, ci = ed_sb, c - HD[0]
                    dst = slice(576 * ci + 192 * k, 576 * ci + 192 * (k + 1))
                    scalar.activation(dst_t[:, dst], src, AF.Exp).then_inc(cp_sem)

                def recip(s_slice, r_slice, accum=None):
                    scalar.activation(ln_sb[:, :], s_sb[:, s_slice], AF.Ln, accum_out=accum)
                    scalar.activation(
                        r_sb[:, r_slice], ln_sb[:, :], AF.Exp, scale=-1.0
                    ).then_inc(rs_sem)

                for c in H1:
                    for k in range(3):
                        exp_copy(c, k)
                scalar.wait_ge(vx_sem, 1)  # S1
                recip(slice(0, 576), slice(0, 576))
                for c in H2:
                    for k in range(3):
                        exp_copy(c, k)
                scalar.wait_ge(vx_sem, 3)  # S2
                recip(slice(576, 1152), slice(576, 1152))
                for c in H0:
                    for k in range(3):
                        exp_copy(c, k)
                scalar.wait_ge(vx_sem, 5)  # S0
                recip(slice(1152, 1728), slice(1152, 1728),
                      accum=acc_sb[:, COL_LNS0 : COL_LNS0 + 1])
                for c in HD:
                    for k in range(3):
                        exp_copy(c, k)
                # scalar-owned hist
                scalar.wait_ge(vx_sem, 4)  # x2
                for c in range(2):
                    base = 40 * (10 + c)
                    xs = xb2_sb[:, 576 * c : 576 * (c + 1)]
                    for j, t in enumerate(thr(K12)):
                        scalar.activation(
                            junk_s[:, :], xs, AF.Relu, bias=-t, scale=-1.0,
                            accum_out=acc_sb[:, base + j : base + j + 1],
                        )
                for c in range(7):
                    scalar.wait_ge(vx_sem, 6 + c)  # x0 class c ready
                    base = 40 * c
                    xs = xb0_sb[:, 576 * c : 576 * (c + 1)]
                    for j, t in enumerate(thr(K0)):
                        scalar.activation(
                            junk_s[:, :], xs, AF.Relu, bias=-t, scale=-1.0,
                            accum_out=acc_sb[:, base + j : base + j + 1],
                        )
                    if c >= 4:
                        continue  # TF of classes 4-6 run on vector
                    for j, t in enumerate(thr(K0)):
                        scalar.activation(
                            junk_s[:, :], xs, AF.Relu, bias=-t, scale=1.0,
                            accum_out=acc_sb[:, base + 20 + j : base + 21 + j],
                        )
                scalar.wait_ge(vx_sem, 13)  # Sd
                scalar.activation(
                    junk_s[:, :], s_sb[:, 1728:2304], AF.Ln,
                    accum_out=acc_sb[:, COL_LNSD : COL_LNSD + 1],
                )
                scalar.activation(junk_s[:, :], onesw_sb[:, :], AF.Copy).then_inc(hist_sem)

            @block.vector
            def _(vector):
                vector.wait_ge(g_sem, 1)
                vector.wait_ge(dmaT, 16)
                tfo = {"x0": 0, "x1": 1, "x2": 2}
                for c in range(NCH):
                    vector.wait_ge(mm_sem, c + 1)
                    vector.tensor_copy(
                        t1_sb[0:96, 192 * c : 192 * (c + 1)],
                        ps1[c % 2][0:96, 0:192],
                    ).then_inc(cp1_sem)
                for (xk, cc, K, base) in CLASSES:
                    h = tfo[xk]
                    vector.scalar_tensor_tensor(
                        junk_v[:, :], tf_sb[:, 576 * h : 576 * (h + 1)], float(cc),
                        onesw_sb[:, :], OP.is_equal, OP.mult,
                        accum_out=acc_sb[:, base + 38 : base + 39],
                    )

                def softmax_x(ek, xbk, srange, rrange, tfrange, C, rs_target):
                    ins2 = vector.tensor_add(s_sb[:, srange], ek[:, 0:576], ek[:, 576:1152])
                    for cc in range(2, C):
                        ins2 = vector.tensor_add(
                            s_sb[:, srange], s_sb[:, srange], ek[:, 576 * cc : 576 * (cc + 1)]
                        )
                    ins2.then_inc(vx_sem)  # S ready
                    vector.wait_ge(rs_sem, rs_target)
                    ins3 = None
                    for cc in range(C):
                        cs = slice(576 * cc, 576 * (cc + 1))
                        vector.tensor_mul(ek[:, cs], ek[:, cs], r_sb[:, rrange])
                        ins3 = vector.scalar_tensor_tensor(
                            xbk[:, cs], tf_sb[:, tfrange], float(cc), ek[:, cs],
                            OP.is_equal, OP.subtract,
                        )
                    ins3.then_inc(vx_sem)  # x ready

                vector.wait_ge(cp_sem, 9)
                softmax_x(e1_sb, xb1_sb, slice(0, 576), slice(0, 576), slice(576, 1152), 3, 1)
                vector.wait_ge(cp_sem, 15)
                softmax_x(e2_sb, xb2_sb, slice(576, 1152), slice(576, 1152), slice(1152, 1728), 2, 2)
                # zfg accums from PSUM for head0 channels
                for c in H0:
                    for k in range(3):
                        idx = 3 * c + k
                        vector.wait_ge(mm_sem, 19 + idx + 1)
                        vector.wait_ge(cp_sem, idx + 1)
                        vector.scalar_tensor_tensor(
                            junk_v[:, 0:192],
                            tf_sb[:, 192 * k : 192 * (k + 1)],
                            float(c - H0[0]),
                            ps2[idx % 5][0:128, 0:192],
                            OP.is_equal, OP.mult,
                            accum_out=acc_sb[:, zcol(c, k) : zcol(c, k) + 1],
                        ).then_inc(cpz_sem)
                vector.wait_ge(cp_sem, 36)
                ins2 = vector.tensor_add(s_sb[:, 1152:1728], e0_sb[:, 0:576], e0_sb[:, 576:1152])
                for cc in range(2, 7):
                    ins2 = vector.tensor_add(
                        s_sb[:, 1152:1728], s_sb[:, 1152:1728], e0_sb[:, 576 * cc : 576 * (cc + 1)]
                    )
                ins2.then_inc(vx_sem)  # vx=5: S0
                vector.wait_ge(rs_sem, 3)
                for cc in range(7):
                    cs = slice(576 * cc, 576 * (cc + 1))
                    mul = vector.tensor_mul(e0_sb[:, cs], e0_sb[:, cs], r_sb[:, 1152:1728])
                    if cc >= 1:
                        mul.then_inc(vx_sem)  # vx=6+cc-1: x0 class cc-1 landed
                    vector.scalar_tensor_tensor(
                        xb0_sb[:, cs], tf_sb[:, 0:576], float(cc), e0_sb[:, cs],
                        OP.is_equal, OP.subtract,
                    )
                vector.memset(junk_v[:, :], 0.0).then_inc(vx_sem)  # vx=12: x0 class 6
                # zfg accums for dsn channels
                for c in HD:
                    for k in range(3):
                        idx = 3 * c + k
                        vector.wait_ge(mm_sem, 19 + idx + 1)
                        vector.wait_ge(cp_sem, idx + 1)
                        vector.scalar_tensor_tensor(
                            junk_v[:, 0:192],
                            tf_sb[:, 192 * k : 192 * (k + 1)],
                            float(c - HD[0]),
                            ps2[idx % 5][0:128, 0:192],
                            OP.is_equal, OP.mult,
                            accum_out=acc_sb[:, zcol(c, k) : zcol(c, k) + 1],
                        ).then_inc(cpz_sem)
                vector.wait_ge(cp_sem, 57)
                ins = vector.tensor_add(s_sb[:, 1728:2304], ed_sb[:, 0:576], ed_sb[:, 576:1152])
                for cc in range(2, 7):
                    ins = vector.tensor_add(
                        s_sb[:, 1728:2304], s_sb[:, 1728:2304], ed_sb[:, 576 * cc : 576 * (cc + 1)]
                    )
                ins.then_inc(vx_sem)  # Sd
                # vector-owned hist: head1 TB (-TB via min) + TF, head2 TF
                for c in range(3):
                    base = 40 * (7 + c)
                    xs = xb1_sb[:, 576 * c : 576 * (c + 1)]
                    for j, t in enumerate(thr(K12)):
                        vector.scalar_tensor_tensor(
                            junk_vb[:, :], xs, t, zerob_sb[:, :], OP.add, OP.min,
                            accum_out=acc_sb[:, base + j : base + j + 1],
                        )
                    for j, t in enumerate(thr(K12)):
                        vector.scalar_tensor_tensor(
                            junk_vb[:, :], xs, t, zerob_sb[:, :], OP.subtract, OP.max,
                            accum_out=acc_sb[:, base + 20 + j : base + 21 + j],
                        )
                for c in range(2):
                    base = 40 * (10 + c)
                    xs = xb2_sb[:, 576 * c : 576 * (c + 1)]
                    for j, t in enumerate(thr(K12)):
                        vector.scalar_tensor_tensor(
                            junk_vb[:, :], xs, t, zerob_sb[:, :], OP.subtract, OP.max,
                            accum_out=acc_sb[:, base + 20 + j : base + 21 + j],
                        )
                for c in (4, 5, 6):
                    base = 40 * c
                    xs = xb0_sb[:, 576 * c : 576 * (c + 1)]
                    for j, t in enumerate(thr(K0)):
                        vector.scalar_tensor_tensor(
                            junk_vb[:, :], xs, t, zerob_sb[:, :], OP.subtract, OP.max,
                            accum_out=acc_sb[:, base + 20 + j : base + 21 + j],
                        )
                vector.memset(junk_v[:, :], 0.0)
                vector.memset(junk_v[:, :], 0.0).then_inc(hist_sem)
                # ---- finale ----
                vector.wait_ge(mm_sem, 77)
                vector.tensor_copy(rst_sb[0:1, :], psR[0:1, :])
                vector.memset(junk_v[:, :], 0.0)
                for i, (xk, c, K, base) in enumerate(CLASSES):
                    if xk == "x1":
                        vector.tensor_sub(
                            tmpA_sb[0:1, 16 * i : 16 * i + K],
                            rst_sb[0:1, base + 1 : base + K + 1],
                            rst_sb[0:1, base : base + K],
                        )
                    else:
                        vector.tensor_sub(
                            tmpA_sb[0:1, 16 * i : 16 * i + K],
                            rst_sb[0:1, base : base + K],
                            rst_sb[0:1, base + 1 : base + K + 1],
                        )
                for i, (xk, c, K, base) in enumerate(CLASSES):
                    vector.tensor_scalar(
                        tmpB_sb[0:1, 16 * i : 16 * i + K],
                        tmpA_sb[0:1, 16 * i : 16 * i + K],
                        float(K), rst_sb[0:1, base + 38 : base + 39],
                        OP.mult, OP.add,
                    )
                for i, (xk, c, K, base) in enumerate(CLASSES):
                    vector.reciprocal(
                        tmpC_sb[0:1, 16 * i : 16 * i + K],
                        tmpB_sb[0:1, 16 * i : 16 * i + K],
                    )
                for i, (xk, c, K, base) in enumerate(CLASSES):
                    vector.tensor_sub(
                        tmpD_sb[0:1, 16 * i : 16 * i + K],
                        rst_sb[0:1, base + 20 : base + 20 + K],
                        rst_sb[0:1, base + 21 : base + 21 + K],
                    )
                for i, (xk, c, K, base) in enumerate(CLASSES):
                    vector.tensor_add(
                        tmpD_sb[0:1, 16 * i : 16 * i + K],
                        tmpD_sb[0:1, 16 * i : 16 * i + K],
                        tmpA_sb[0:1, 16 * i : 16 * i + K],
                    )
                for i, (xk, c, K, base) in enumerate(CLASSES):
                    vector.scalar_tensor_tensor(
                        tmpE_sb[0:1, 16 * i : 16 * i + K],
                        tmpD_sb[0:1, 16 * i : 16 * i + K],
                        1.0, tmpC_sb[0:1, 16 * i : 16 * i + K],
                        OP.mult, OP.mult,
                        accum_out=rst_sb[0:1, base + 39 : base + 40],
                    )
                vector.memset(junk_v[:, :], 0.0)
                vector.tensor_mul(tmpw_sb[0:1, :], rst_sb[0:1, :], wv_sb[0:1, :])
                vector.memset(junk_v[:, :], 0.0)
                vector.tensor_reduce(
                    partial_sb[0:1, 0:1], tmpw_sb[0:1, :],
                    mybir.AxisListType.X, OP.add,
                )
                vector.memset(junk_v[:, :], 0.0)
                vector.memset(junk_v[:, :], 0.0).then_inc(fin_sem)

    return nc


# ---------------------------------------------------------------- host side --

def _interp_weights():
    s = np.linspace(np.float32(0.0), np.float32(95.0), 384).astype(np.float32)
    i0 = np.clip(np.floor(s).astype(np.int64), 0, 94)
    t = (s - i0).astype(np.float32)
    return i0, t


def _prep_core(inputs, core):
    b, half = core // 2, core % 2
    r0 = half * 192
    cy0 = 0 if half == 0 else 47
    i0, t = _interp_weights()

    uyt = np.zeros((49, 192), np.float32)
    for fy in range(192):
        f = r0 + fy
        uyt[i0[f] - cy0, fy] += np.float32(1.0) - t[f]
        uyt[i0[f] + 1 - cy0, fy] += t[f]

    ux = np.zeros((96, 384), np.float32)
    for X in range(384):
        ux[i0[X], X] += np.float32(1.0) - t[X]
        ux[i0[X] + 1, X] += t[X]

    heads = [inputs["preds1"], inputs["preds2"], inputs["preds0"], inputs["preds_dsn"]]
    pa = np.zeros((49, NCH * 96), np.float32)
    idx = 0
    for arr in heads:
        for ch in range(arr.shape[1]):
            pa[:, idx * 96 : (idx + 1) * 96] = arr[b, ch, cy0 : cy0 + 49, :]
            idx += 1

    tg = np.zeros((128, 3 * 576), np.float32)
    for h, key in enumerate(["targets0", "targets1", "targets2"]):
        th = inputs[key][b, r0 : r0 + 192, :]
        tg[:, 576 * h : 576 * (h + 1)] = (
            th.reshape(192, 3, 128).transpose(2, 1, 0).reshape(128, 576)
        ).astype(np.float32)

    wv = np.zeros((1, ACC_W), np.float32)
    for (xk, c, K, base) in CLASSES:
        if xk == "x0":
            wv[0, base + 39] = (1.0 / 7.0) / 8.0
        elif xk == "x1":
            wv[0, base + 39] = (0.4 / 3.0) / 8.0
        else:
            wv[0, base + 39] = (0.4 / 2.0) / 8.0
    for ci in range(7):
        wv[0, 40 * ci + 8 : 40 * ci + 11] = -1.0 / P_GLOBAL
        wv[0, 40 * ci + 12 : 40 * ci + 15] = -0.4 / P_GLOBAL
    wv[0, COL_LNS0] = 1.0 / P_GLOBAL
    wv[0, COL_LNSD] = 0.4 / P_GLOBAL

    return {"preds_all": pa, "uyt": uyt, "ux": ux, "tgts": tg, "wvec": wv}


_NC_CACHE = None


def kernel(**inputs):
    global _NC_CACHE
    inputs = {k: np.asarray(v) for k, v in inputs.items()}
    if _NC_CACHE is None:
        _NC_CACHE = build_kernel()
    nc = _NC_CACHE
    in_maps = [_prep_core(inputs, core) for core in range(8)]
    res = run_bass_kernel_spmd(nc, in_maps, core_ids=list(range(8)))
    out = np.asarray(res.results[0]["out"], dtype=np.float32).reshape(-1)
    return np.asarray(out[0], dtype=np.float32)



# revision 7
# speedup vs baseline: 1.5734x; 1.5734x over previous
"""Trainium2 distributed kernel for ABRLovaszCELoss (8 NeuronCores).

Strategy (v4)
-------------
Data-parallel over (batch, row-half): core i handles batch b=i//2, fine rows
[192*(i%2), 192*(i%2)+192) of the 384x384 target grid (73728 pixels/core).

Per core, fully on-device (all-bf16 datapath):
- bilinear align_corners upsample 96->384 of all 19 logit channels
  (order head1:3, head2:2, head0:7, dsn:7) as two bf16 PE matmuls per
  channel; pixel layout [128 part = X%128, 576 free = 192*(X//128) + fy].
- stage-1 PSUM pairs copied to SBUF bf16 by vector; stage-2 pairs exp'd
  384-wide on scalar into one contiguous e_all tile.
- softmax: per-head S sums on vector (dsn S on pool), r = exp(-ln S) on
  scalar; the head0 Ln pass also accumulates CE's sum(ln S); p = e*r
  in-place; x = fg - p; fg masks via fast tensor_scalar is_equal.
- CE's sum(z*[tgt==c]) via linearity of the interpolation:
  <t1_c, A_c> where A_c = ux^T-adjoint of the fg mask (3 accumulating PE
  matmuls per head0 class), consumed by tiny [96,192] stt accumulations.
  n_c is computed exactly on the host from the integer targets.
- Lovasz-Softmax per shard via exact relu tail-integrals (no sort):
  TF_j = sum relu(x - t_j), TB_j = sum relu(-x - t_j) at bf16-exact
  thresholds; per-segment integrals by differencing on the host, and
  L_c = sum_j (IF_j + IB_j) / (n_c + IB_j/dt_j).
  Histogram passes split across vector (tensor_scalar add-reduce accum),
  scalar (Relu+accum), and pool (vector-prepped relu tiles + XYZWC
  tensor_reduce).
- the [128,256] accumulator tile is DMA'd out per core; the final
  per-class differencing/reciprocal algebra and the 8-shard reduction
  happen on the host during gather/unshard (exact fp64).
"""

import numpy as np
import ml_dtypes

import concourse.bass as bass
import concourse.mybir as mybir
from concourse.bass_utils import run_bass_kernel_spmd

F32 = mybir.dt.float32
BF16 = mybir.dt.bfloat16
AF = mybir.ActivationFunctionType
OP = mybir.AluOpType
AX = mybir.AxisListType
BF = ml_dtypes.bfloat16

NCH = 19
N_PIX = 73728
P_GLOBAL = 4 * 384 * 384

# channel order: head1 (3), head2 (2), head0 (7), dsn (7)
THR12 = (0.0, 0.34375, 0.671875)   # bf16-exact ~ j/3
THR0 = (0.0, 0.5)

# lovasz classes in "CL" order
CL = ([("h1", c) for c in range(3)] + [("h2", c) for c in range(2)]
      + [("h0", c) for c in range(7)])
HEAD_CH0 = {"h1": 0, "h2": 3, "h0": 5, "d": 12}
S_OFF = {"h1": 0, "h2": 576, "h0": 1152, "d": 1728}
R_OFF = {"h1": 0, "h2": 576, "h0": 1152}


def chan_of(ci):
    head, c = CL[ci]
    return HEAD_CH0[head] + c


def thr_of(ci):
    return THR12 if ci < 5 else THR0


# histogram pass assignment (side 'B' = TB via min/relu(-x-t), 'F' = TF)
POOL_PASSES = []
for ci in range(4):                       # h1_0..2, h2_0: all 6 passes
    K = len(thr_of(ci))
    POOL_PASSES += [(ci, 'B', j) for j in range(K)]
    POOL_PASSES += [(ci, 'F', j) for j in range(K)]
POOL_PASSES += [(4, 'B', j) for j in range(3)]          # h2_1 TB
V_PASSES = ([(4, 'F', j) for j in range(3)]             # h2_1 TF
            + [(10, s, j) for s in 'BF' for j in range(2)]
            + [(11, s, j) for s in 'BF' for j in range(2)])
S_PASSES = [(ci, s, j) for ci in range(5, 10) for s in 'BF' for j in range(2)]
SCALAR_SET = set(S_PASSES)

ACC_W = 256
COL_LNS0 = 192
COL_LNSD = 193


def col_of(ci, side, j):
    return 16 * ci + (j if side == 'B' else 6 + j)


def col_zf(ci, which):   # which: 0 = head0, 1 = dsn
    return 16 * ci + 13 + which


N_JR = 6   # jr ring slots


def build_kernel():
    nc = bass.Bass()

    # activation bias consts for the scalar-engine hist passes
    bias_vals = sorted({-t for t in THR0[1:]} | {-t for t in THR12[1:]})
    for i, val in enumerate(bias_vals):
        tns = nc.alloc_sbuf_tensor(f"const-thr-{i}", [128, 1], F32)
        nc.gpsimd.memset(tns.ap(), val)
        nc.const_aps.aps[(F32, val)] = tns.ap()
    nc.all_engine_barrier()

    p_preds = nc.declare_dram_parameter("preds", [49, NCH * 96], BF16, isOutput=False)
    p_uyt = nc.declare_dram_parameter("uyt", [49, 192], BF16, isOutput=False)
    p_ux = nc.declare_dram_parameter("ux", [96, 384], BF16, isOutput=False)
    p_uxT = nc.declare_dram_parameter("uxT", [128, 3 * 96], BF16, isOutput=False)
    p_tgt = nc.declare_dram_parameter("tgt", [128, 3 * 576], BF16, isOutput=False)
    p_acc = nc.declare_dram_parameter("acc", [128, ACC_W], F32, isOutput=True)

    # ---------------- static program-order op lists (for cross-engine idx) --
    # tensor ops
    tops = []
    for c in range(NCH):
        tops.append(('mm1', c))
    for q in range(29):
        for m in (2 * q, 2 * q + 1):
            if m <= 56:
                tops.append(('mm2', m))
        # A(ci) inserted late (after pair 15+ci) so its wait on V's zfg
        # consumption can never stall stage-2 pairs that gate the exps the
        # V-side softmax chain needs (deadlock-free: V zfg waits only
        # tensor pairs <= 17).
        ci = q - 15
        if 5 <= ci <= 11:
            for k in range(3):
                tops.append(('A', ci, k))
    # vector ops
    vops = []
    vops.append(('copy1', 0))
    vops += [('fg', i) for i in range(4)]
    vops.append(('copy1', 1))
    vops += [('fg', i) for i in range(4, 8)]
    vops.append(('copy1', 2))
    vops += [('fg', i) for i in range(8, 12)]
    vops += [('copy1', j) for j in range(3, 10)]
    vops += [('Sadd', 'h1', 0), ('Sadd', 'h1', 1), ('Sadd', 'h2', 0)]
    vops += [('p', ci) for ci in range(3)]
    vops += [('x', ci) for ci in range(3)]
    vops += [('jr', n) for n in range(18)]            # h1 preps
    vops += [('p', 3), ('p', 4), ('x', 3), ('x', 4)]
    vops += [('jr', n) for n in range(18, 27)]        # h2_0 + h2_1 B preps
    vops += [('histv', 0), ('histv', 1), ('histv', 2)]  # h2_1 F
    vops += [('Sadd', 'h0', i) for i in range(6)]
    vops += [('p', ci) for ci in range(5, 12)]
    vops += [('x', ci) for ci in range(5, 12)]
    vops += [('histv', i) for i in range(3, 11)]      # h0_5, h0_6
    for ci in range(5, 12):
        vops += [('zfh', ci), ('zfd', ci)]
    vops += [('dsnSv', i) for i in range(6)]
    # scalar ops
    sops = []
    for q in range(29):
        sops.append(('exp', q))
        if q == 5:
            sops += [('ln', 'h1'), ('rexp', 'h1')]
        if q == 8:
            sops += [('ln', 'h2'), ('rexp', 'h2')]
        if q == 18:
            sops += [('ln', 'h0'), ('rexp', 'h0')]
    sops += [('hists', n) for n in range(len(S_PASSES))]
    sops.append(('lnd',))
    # pool ops
    pops = [('memset',)]
    pops += [('red', n) for n in range(27)]

    tidx = {op: i + 1 for i, op in enumerate(tops)}
    vidx = {op: i + 1 for i, op in enumerate(vops)}
    sidx = {op: i + 1 for i, op in enumerate(sops)}
    pidx = {op: i + 1 for i, op in enumerate(pops)}

    from contextlib import ExitStack
    with ExitStack() as es:
        def sb(name, shape, dtype=F32):
            return es.enter_context(nc.sbuf_tensor(name, shape, dtype))

        preds_sb = sb("preds_sb", [49, NCH * 96], BF16)
        uyt_sb = sb("uyt_sb", [49, 192], BF16)
        ux_sb = sb("ux_sb", [96, 384], BF16)
        uxT_sb = sb("uxT_sb", [128, 3 * 96], BF16)
        tf_sb = sb("tf_sb", [128, 3 * 576], BF16)
        t1_sb = sb("t1_sb", [96, NCH * 192], BF16)
        e_sb = sb("e_sb", [128, NCH * 576], BF16)
        s_sb = sb("s_sb", [128, 4 * 576], BF16)
        r_sb = sb("r_sb", [128, 3 * 576], BF16)
        ln_sb = sb("ln_sb", [128, 576])
        fg_sb = sb("fg_sb", [128, 12 * 576], BF16)
        xb_sb = sb("xb_sb", [128, 12 * 576], BF16)
        jr_sb = sb("jr_sb", [128, N_JR * 576], BF16)
        junkv_sb = sb("junkv_sb", [128, 576], BF16)
        junks_sb = sb("junks_sb", [128, 576], BF16)
        acc_sb = sb("acc_sb", [128, ACC_W])

        ps1 = [es.enter_context(nc.psum_tensor(f"ps1{i}", [96, 384], F32)) for i in range(3)]
        ps2 = [es.enter_context(nc.psum_tensor(f"ps2{i}", [128, 384], F32)) for i in range(3)]
        psA = [es.enter_context(nc.psum_tensor(f"psA{i}", [96, 192], F32)) for i in range(2)]

        dmaP = es.enter_context(nc.semaphore("dmaP"))
        dmaU = es.enter_context(nc.semaphore("dmaU"))
        dmaX = es.enter_context(nc.semaphore("dmaX"))
        dmaXT = es.enter_context(nc.semaphore("dmaXT"))
        dmaT = es.enter_context(nc.semaphore("dmaT"))
        t_sem = es.enter_context(nc.semaphore("t_sem"))
        v_sem = es.enter_context(nc.semaphore("v_sem"))
        s_sem = es.enter_context(nc.semaphore("s_sem"))
        p_sem = es.enter_context(nc.semaphore("p_sem"))
        odma = es.enter_context(nc.semaphore("odma"))

        SEMS = {'t': t_sem, 'v': v_sem, 's': s_sem, 'p': p_sem,
                'P': dmaP, 'U': dmaU, 'X': dmaX, 'XT': dmaXT, 'T': dmaT}
        IDX = {'t': tidx, 'v': vidx, 's': sidx, 'p': pidx}

        def mk_waiter(eng):
            seen = {}
            def wait(dom, tag=None):
                sem = SEMS[dom]
                n = 16 if tag is None else IDX[dom][tag]
                if seen.get(dom, 0) >= n:
                    return
                seen[dom] = n
                eng.wait_ge(sem, n)
            return wait

        # slice helpers
        def e_ch(c):
            return e_sb[:, 576 * c: 576 * (c + 1)]

        def t1_ch(c):
            return t1_sb[0:96, 192 * c: 192 * (c + 1)]

        def fg_t(ci):
            return fg_sb[:, 576 * ci: 576 * (ci + 1)]

        def xb_t(ci):
            return xb_sb[:, 576 * ci: 576 * (ci + 1)]

        def s_t(h):
            return s_sb[:, S_OFF[h]: S_OFF[h] + 576]

        def r_t(h):
            return r_sb[:, R_OFF[h]: R_OFF[h] + 576]

        def jr_t(n):
            s = n % N_JR
            return jr_sb[:, 576 * s: 576 * (s + 1)]

        def tf_head(ci):
            head = CL[ci][0]
            off = {"h0": 0, "h1": 576, "h2": 1152}[head]
            return tf_sb[:, off: off + 576]

        def acc_col(col, rows=128):
            return acc_sb[0:rows, col: col + 1]

        # exp bank boundary helpers
        def expbank_of_chunk(m):
            return m // 2

        def e_ready_bank(c):
            """exp bank index that completes channel c's tile."""
            return expbank_of_chunk(3 * c + 2)

        with nc.Block() as block:

            @block.sync
            def _(sync):
                sync.dma_start(out=preds_sb[:, :], in_=p_preds[:, :]).then_inc(dmaP, 16)
                sync.dma_start(out=uyt_sb[:, :], in_=p_uyt[:, :]).then_inc(dmaU, 16)
                sync.dma_start(out=ux_sb[:, :], in_=p_ux[:, :]).then_inc(dmaX, 16)
                sync.dma_start(out=uxT_sb[:, :], in_=p_uxT[:, :]).then_inc(dmaXT, 16)
                sync.dma_start(out=tf_sb[:, :], in_=p_tgt[:, :]).then_inc(dmaT, 16)
                sync.wait_ge(v_sem, len(vops))
                sync.wait_ge(s_sem, len(sops))
                sync.wait_ge(p_sem, len(pops))
                sync.dma_start(out=p_acc[:, :], in_=acc_sb[:, :]).then_inc(odma, 16)
                sync.wait_ge(odma, 16)

            @block.tensor
            def _(tensor):
                wait = mk_waiter(tensor)
                for op in tops:
                    if op[0] == 'mm1':
                        c = op[1]
                        if c == 0:
                            wait('P'); wait('U')
                        j = c // 2
                        if c % 2 == 0 and j >= 3:
                            wait('v', ('copy1', j - 3))
                        tensor.matmul(
                            ps1[j % 3][0:96, 192 * (c % 2): 192 * (c % 2) + 192],
                            preds_sb[0:49, 96 * c: 96 * (c + 1)],
                            uyt_sb[0:49, 0:192],
                            start=True, stop=True,
                        ).then_inc(t_sem)
                    elif op[0] == 'mm2':
                        m = op[1]
                        c, k = divmod(m, 3)
                        q = m // 2
                        if m == 0:
                            wait('X')
                        wait('v', ('copy1', c // 2))
                        if q >= 3 and m % 2 == 0:
                            wait('s', ('exp', q - 3))
                        tensor.matmul(
                            ps2[q % 3][0:128, 192 * (m % 2): 192 * (m % 2) + 192],
                            ux_sb[0:96, 128 * k: 128 * (k + 1)],
                            t1_ch(c),
                            start=True, stop=True,
                        ).then_inc(t_sem)
                    else:  # A matmul
                        _, ci, k = op
                        if k == 0:
                            wait('XT')
                            wait('v', ('fg', ci))
                            if ci >= 7:
                                wait('v', ('zfd', ci - 2))
                        tensor.matmul(
                            psA[ci % 2][0:96, 0:192],
                            uxT_sb[0:128, 96 * k: 96 * (k + 1)],
                            fg_sb[:, 576 * ci + 192 * k: 576 * ci + 192 * (k + 1)],
                            start=(k == 0), stop=(k == 2),
                        ).then_inc(t_sem)

            @block.scalar
            def _(scalar):
                wait = mk_waiter(scalar)
                for op in sops:
                    if op[0] == 'exp':
                        q = op[1]
                        w = 384 if q < 28 else 192
                        wait('t', ('mm2', min(2 * q + 1, 56)))
                        scalar.activation(
                            e_sb[:, 384 * q: 384 * q + w],
                            ps2[q % 3][0:128, 0:w], AF.Exp,
                        ).then_inc(s_sem)
                    elif op[0] == 'ln':
                        h = op[1]
                        if h == 'h1':
                            wait('v', ('Sadd', 'h1', 1))
                            scalar.activation(ln_sb[:, :], s_t('h1'), AF.Ln).then_inc(s_sem)
                        elif h == 'h2':
                            wait('v', ('Sadd', 'h2', 0))
                            scalar.activation(ln_sb[:, :], s_t('h2'), AF.Ln).then_inc(s_sem)
                        else:
                            wait('v', ('Sadd', 'h0', 5))
                            wait('p', ('memset',))
                            scalar.activation(
                                ln_sb[:, :], s_t('h0'), AF.Ln,
                                accum_out=acc_col(COL_LNS0),
                            ).then_inc(s_sem)
                    elif op[0] == 'rexp':
                        h = op[1]
                        scalar.activation(r_t(h), ln_sb[:, :], AF.Exp, scale=-1.0).then_inc(s_sem)
                    elif op[0] == 'lnd':
                        wait('v', ('dsnSv', 5))
                        scalar.activation(
                            junks_sb[:, :], s_t('d'), AF.Ln,
                            accum_out=acc_col(COL_LNSD),
                        ).then_inc(s_sem)
                    else:  # hists
                        n = op[1]
                        ci, side, j = S_PASSES[n]
                        t = thr_of(ci)[j]
                        wait('v', ('x', ci))
                        scalar.activation(
                            junks_sb[:, :], xb_t(ci), AF.Relu,
                            bias=-t, scale=(1.0 if side == 'F' else -1.0),
                            accum_out=acc_col(col_of(ci, side, j)),
                        ).then_inc(s_sem)

            @block.vector
            def _(vector):
                wait = mk_waiter(vector)
                first_fg = True
                first_acc = True
                for op in vops:
                    if op[0] == 'copy1':
                        j = op[1]
                        w = 384 if j < 9 else 192
                        wait('t', ('mm1', min(2 * j + 1, 18)))
                        vector.tensor_copy(
                            t1_sb[0:96, 384 * j: 384 * j + w],
                            ps1[j % 3][0:96, 0:w],
                        ).then_inc(v_sem)
                    elif op[0] == 'fg':
                        ci = op[1]
                        if first_fg:
                            wait('T')
                            first_fg = False
                        head, c = CL[ci]
                        vector.tensor_scalar(
                            fg_t(ci), tf_head(ci), float(c), 0.0,
                            OP.is_equal, OP.add,
                        ).then_inc(v_sem)
                    elif op[0] == 'Sadd':
                        _, h, i = op
                        if h == 'h1':
                            if i == 0:
                                wait('s', ('exp', e_ready_bank(1)))
                                vector.tensor_add(s_t('h1'), e_ch(0), e_ch(1)).then_inc(v_sem)
                            else:
                                wait('s', ('exp', e_ready_bank(2)))
                                vector.tensor_add(s_t('h1'), s_t('h1'), e_ch(2)).then_inc(v_sem)
                        elif h == 'h2':
                            wait('s', ('exp', e_ready_bank(4)))
                            vector.tensor_add(s_t('h2'), e_ch(3), e_ch(4)).then_inc(v_sem)
                        else:
                            if i == 0:
                                wait('s', ('exp', e_ready_bank(6)))
                                vector.tensor_add(s_t('h0'), e_ch(5), e_ch(6)).then_inc(v_sem)
                            else:
                                wait('s', ('exp', e_ready_bank(6 + i)))
                                vector.tensor_add(s_t('h0'), s_t('h0'), e_ch(6 + i)).then_inc(v_sem)
                    elif op[0] == 'p':
                        ci = op[1]
                        head = CL[ci][0]
                        wait('s', ('rexp', head))
                        ch = chan_of(ci)
                        vector.tensor_mul(e_ch(ch), e_ch(ch), r_t(head)).then_inc(v_sem)
                    elif op[0] == 'x':
                        ci = op[1]
                        vector.tensor_tensor(
                            xb_t(ci), fg_t(ci), e_ch(chan_of(ci)), OP.subtract,
                        ).then_inc(v_sem)
                    elif op[0] == 'jr':
                        n = op[1]
                        ci, side, j = POOL_PASSES[n]
                        t = thr_of(ci)[j]
                        if n >= N_JR:
                            wait('p', ('red', n - N_JR))
                        if side == 'F':
                            vector.tensor_scalar(jr_t(n), xb_t(ci), t, 0.0,
                                                 OP.max, OP.add).then_inc(v_sem)
                        else:
                            vector.tensor_scalar(jr_t(n), xb_t(ci), -t, 0.0,
                                                 OP.min, OP.add).then_inc(v_sem)
                    elif op[0] == 'histv':
                        n = op[1]
                        ci, side, j = V_PASSES[n]
                        t = thr_of(ci)[j]
                        if first_acc:
                            wait('p', ('memset',))
                            first_acc = False
                        cl = acc_col(col_of(ci, side, j))
                        if side == 'F':
                            vector.tensor_scalar(junkv_sb[:, :], xb_t(ci), t, 0.0,
                                                 OP.max, OP.add, accum_out=cl).then_inc(v_sem)
                        else:
                            vector.tensor_scalar(junkv_sb[:, :], xb_t(ci), -t, 0.0,
                                                 OP.min, OP.add, accum_out=cl).then_inc(v_sem)
                    elif op[0] == 'zfh':
                        ci = op[1]
                        wait('t', ('A', ci, 2))
                        if first_acc:
                            wait('p', ('memset',))
                            first_acc = False
                        vector.scalar_tensor_tensor(
                            junkv_sb[0:96, 0:192], t1_ch(ci), 1.0,
                            psA[ci % 2][0:96, 0:192], OP.mult, OP.mult,
                            accum_out=acc_col(col_zf(ci, 0), rows=96),
                        ).then_inc(v_sem)
                    elif op[0] == 'zfd':
                        ci = op[1]
                        vector.scalar_tensor_tensor(
                            junkv_sb[0:96, 0:192], t1_ch(ci + 7), 1.0,
                            psA[ci % 2][0:96, 0:192], OP.mult, OP.mult,
                            accum_out=acc_col(col_zf(ci, 1), rows=96),
                        ).then_inc(v_sem)
                    else:  # dsnSv
                        i = op[1]
                        if i == 0:
                            wait('s', ('exp', e_ready_bank(13)))
                            vector.tensor_add(s_t('d'), e_ch(12), e_ch(13)).then_inc(v_sem)
                        else:
                            wait('s', ('exp', e_ready_bank(13 + i)))
                            vector.tensor_add(s_t('d'), s_t('d'), e_ch(13 + i)).then_inc(v_sem)

            @block.gpsimd
            def _(gpsimd):
                wait = mk_waiter(gpsimd)
                for op in pops:
                    if op[0] == 'memset':
                        gpsimd.memset(acc_sb[:, :], 0.0).then_inc(p_sem)
                    elif op[0] == 'red':
                        n = op[1]
                        ci, side, j = POOL_PASSES[n]
                        wait('v', ('jr', n))
                        gpsimd.tensor_reduce(
                            acc_sb[0:1, col_of(ci, side, j): col_of(ci, side, j) + 1],
                            jr_t(n), AX.XYZWC, OP.add,
                        ).then_inc(p_sem)


    return nc


# ---------------------------------------------------------------- host side --

def _interp_weights():
    s = np.linspace(np.float32(0.0), np.float32(95.0), 384).astype(np.float32)
    i0 = np.clip(np.floor(s).astype(np.int64), 0, 94)
    t = (s - i0).astype(np.float32)
    return i0, t


_CHAN_SRC = ([("preds1", c) for c in range(3)] + [("preds2", c) for c in range(2)]
             + [("preds0", c) for c in range(7)] + [("preds_dsn", c) for c in range(7)])


def _prep_core(inputs, core):
    b, half = core // 2, core % 2
    r0 = half * 192
    cy0 = 0 if half == 0 else 47
    i0, t = _interp_weights()

    uyt = np.zeros((49, 192), np.float32)
    for fy in range(192):
        f = r0 + fy
        uyt[i0[f] - cy0, fy] += np.float32(1.0) - t[f]
        uyt[i0[f] + 1 - cy0, fy] += t[f]

    ux = np.zeros((96, 384), np.float32)
    for X in range(384):
        ux[i0[X], X] += np.float32(1.0) - t[X]
        ux[i0[X] + 1, X] += t[X]
    ux = ux.astype(BF)
    uxT = np.zeros((128, 3 * 96), BF)
    for k in range(3):
        uxT[:, 96 * k: 96 * (k + 1)] = ux[:, 128 * k: 128 * (k + 1)].T

    pa = np.zeros((49, NCH * 96), BF)
    for idx, (key, ch) in enumerate(_CHAN_SRC):
        pa[:, idx * 96: (idx + 1) * 96] = inputs[key][b, ch, cy0: cy0 + 49, :].astype(BF)

    tg = np.zeros((128, 3 * 576), BF)
    for h, key in enumerate(["targets0", "targets1", "targets2"]):
        th = inputs[key][b, r0: r0 + 192, :]
        tg[:, 576 * h: 576 * (h + 1)] = (
            th.reshape(192, 3, 128).transpose(2, 1, 0).reshape(128, 576)
        ).astype(BF)

    return {"preds": pa, "uyt": uyt.astype(BF), "ux": ux, "uxT": uxT, "tgt": tg}


def _ncs_core(inputs, core):
    """Exact per-class pixel counts for this shard, from integer targets."""
    b, half = core // 2, core % 2
    r0 = half * 192
    ncs = []
    for ci, (head, c) in enumerate(CL):
        key = {"h1": "targets1", "h2": "targets2", "h0": "targets0"}[head]
        lab = inputs[key][b, r0: r0 + 192, :]
        ncs.append(float((lab == c).sum()))
    return ncs


def _finale(accs, ncs_all):
    lov_total = 0.0
    ce0_num = 0.0
    ced_num = 0.0
    for acc, ncs in zip(accs, ncs_all):
        cs = acc.astype(np.float64).sum(axis=0)
        head_lov = {"h1": [], "h2": [], "h0": []}
        for ci, (head, c) in enumerate(CL):
            thr = thr_of(ci)
            K = len(thr)
            base = 16 * ci
            n_c = ncs[ci]
            TF, TB = [], []
            for j, t in enumerate(thr):
                cF = cs[base + 6 + j]
                cB = cs[base + j]
                if (ci, 'F', j) in SCALAR_SET:
                    TF.append(cF)
                else:
                    TF.append(cF - N_PIX * t)
                if (ci, 'B', j) in SCALAR_SET:
                    TB.append(cB)
                else:
                    TB.append(-cB - N_PIX * t)
            TF.append(0.0)
            TB.append(0.0)
            if n_c < 0.5:
                continue
            ts_ext = list(thr) + [1.0]
            L = 0.0
            for j in range(K):
                IF = TF[j] - TF[j + 1]
                IB = TB[j] - TB[j + 1]
                d = ts_ext[j + 1] - ts_ext[j]
                L += (IF + IB) / (n_c + IB / d)
            head_lov[head].append(L)
        for head, w in (("h0", 1.0), ("h1", 0.4), ("h2", 0.4)):
            vals = head_lov[head]
            lov_total += w * (sum(vals) / max(len(vals), 1))
        ce0_num += cs[COL_LNS0] - sum(cs[16 * ci + 13] for ci in range(5, 12))
        ced_num += cs[COL_LNSD] - sum(cs[16 * ci + 14] for ci in range(5, 12))
    return ce0_num / P_GLOBAL + 0.4 * (ced_num / P_GLOBAL) + lov_total / 8.0


_NC_CACHE = None


def kernel(**inputs):
    global _NC_CACHE
    inputs = {k: np.asarray(v) for k, v in inputs.items()}
    if _NC_CACHE is None:
        _NC_CACHE = build_kernel()
    nc = _NC_CACHE
    in_maps = [_prep_core(inputs, core) for core in range(8)]
    res = run_bass_kernel_spmd(nc, in_maps, core_ids=list(range(8)))
    accs = [np.asarray(res.results[c]["acc"], dtype=np.float32) for c in range(8)]
    ncs_all = [_ncs_core(inputs, c) for c in range(8)]
    loss = _finale(accs, ncs_all)
    return np.asarray(loss, dtype=np.float32)


# revision 10
# speedup vs baseline: 2.4495x; 1.5568x over previous
"""Trainium2 distributed kernel for ABRLovaszCELoss (8 NeuronCores).

Strategy (v4)
-------------
Data-parallel over (batch, row-half): core i handles batch b=i//2, fine rows
[192*(i%2), 192*(i%2)+192) of the 384x384 target grid (73728 pixels/core).

Per core, fully on-device (all-bf16 datapath):
- bilinear align_corners upsample 96->384 of all 19 logit channels
  (order head1:3, head2:2, head0:7, dsn:7) as two bf16 PE matmuls per
  channel; pixel layout [128 part = X%128, 576 free = 192*(X//128) + fy].
- stage-1 PSUM pairs copied to SBUF bf16 by vector; stage-2 pairs exp'd
  384-wide on scalar into one contiguous e_all tile.
- softmax: per-head S sums on vector (dsn S on pool), r = exp(-ln S) on
  scalar; the head0 Ln pass also accumulates CE's sum(ln S); p = e*r
  in-place; x = fg - p; fg masks via fast tensor_scalar is_equal.
- CE's sum(z*[tgt==c]) via linearity of the interpolation:
  <t1_c, A_c> where A_c = ux^T-adjoint of the fg mask (3 accumulating PE
  matmuls per head0 class), consumed by tiny [96,192] stt accumulations.
  n_c is computed exactly on the host from the integer targets.
- Lovasz-Softmax per shard via exact relu tail-integrals (no sort):
  TF_j = sum relu(x - t_j), TB_j = sum relu(-x - t_j) at bf16-exact
  thresholds; per-segment integrals by differencing on the host, and
  L_c = sum_j (IF_j + IB_j) / (n_c + IB_j/dt_j).
  Histogram passes split across vector (tensor_scalar add-reduce accum),
  scalar (Relu+accum), and pool (vector-prepped relu tiles + XYZWC
  tensor_reduce).
- the [128,256] accumulator tile is DMA'd out per core; the final
  per-class differencing/reciprocal algebra and the 8-shard reduction
  happen on the host during gather/unshard (exact fp64).
"""

import numpy as np
import ml_dtypes

import concourse.bass as bass
import concourse.mybir as mybir
from concourse.bass_utils import run_bass_kernel_spmd

F32 = mybir.dt.float32
BF16 = mybir.dt.bfloat16
AF = mybir.ActivationFunctionType
OP = mybir.AluOpType
AX = mybir.AxisListType
BF = ml_dtypes.bfloat16

NCH = 19
N_PIX = 73728
P_GLOBAL = 4 * 384 * 384

# channel order: head1 (3), head2 (2), head0 (7), dsn (7)
THR12 = (0.0, 0.34375, 0.671875)   # bf16-exact ~ j/3
THR0 = (0.0, 0.5)

# lovasz classes in "CL" order
CL = ([("h1", c) for c in range(3)] + [("h2", c) for c in range(2)]
      + [("h0", c) for c in range(7)])
HEAD_CH0 = {"h1": 0, "h2": 3, "h0": 5, "d": 12}
S_OFF = {"h1": 0, "h2": 576, "h0": 1152, "d": 1728}
R_OFF = {"h1": 0, "h2": 576, "h0": 1152}


def chan_of(ci):
    head, c = CL[ci]
    return HEAD_CH0[head] + c


def thr_of(ci):
    return THR12 if ci < 5 else THR0


# histogram pass assignment (side 'B' = TB via min/relu(-x-t), 'F' = TF)
# vector: h1 + h2 classes (tensor_scalar add-reduce); scalar: all h0 classes
V_PASSES = [(ci, s, j) for ci in range(5) for s in 'BF' for j in range(3)]
S_PASSES = [(ci, s, j) for ci in range(5, 12) for s in 'BF' for j in range(2)]
SCALAR_SET = set(S_PASSES)

ACC_W = 256
COL_LNS0 = 192
COL_LNSD = 193


def col_of(ci, side, j):
    return 16 * ci + (j if side == 'B' else 6 + j)


def col_zf(ci, which):   # which: 0 = head0, 1 = dsn
    return 16 * ci + 13 + which


N_JR = 6   # jr ring slots


BIAS_VALS = sorted({-t for t in THR0[1:]} | {-t for t in THR12[1:]})


def build_kernel():
    nc = bass.Bass()

    p_cst = nc.declare_dram_parameter("cst", [128, 4], F32, isOutput=False)
    p_preds = nc.declare_dram_parameter("preds", [49, NCH * 96], BF16, isOutput=False)
    p_uyt = nc.declare_dram_parameter("uyt", [49, 192], BF16, isOutput=False)
    p_ux = nc.declare_dram_parameter("ux", [96, 384], BF16, isOutput=False)
    p_uxT = nc.declare_dram_parameter("uxT", [128, 3 * 96], BF16, isOutput=False)
    p_tgt = nc.declare_dram_parameter("tgt", [128, 3 * 576], BF16, isOutput=False)
    p_acc = nc.declare_dram_parameter("acc", [128, ACC_W], F32, isOutput=True)

    # ---------------- static program-order op lists (for cross-engine idx) --
    # tensor ops
    tops = []
    for c in range(NCH):
        tops.append(('mm1', c))
    for q in range(29):
        for m in (2 * q, 2 * q + 1):
            if m <= 56:
                tops.append(('mm2', m))
        # A(ci) inserted late (after pair 15+ci) so its wait on V's zfg
        # consumption can never stall stage-2 pairs that gate the exps the
        # V-side softmax chain needs (deadlock-free: V zfg waits only
        # tensor pairs <= 17).
        ci = q - 15
        if 5 <= ci <= 11:
            for k in range(3):
                tops.append(('A', ci, k))
    # vector ops
    vops = []
    vops.append(('copy1', 0))
    vops += [('fg', i) for i in range(4)]
    vops.append(('copy1', 1))
    vops += [('fg', i) for i in range(4, 8)]
    vops.append(('copy1', 2))
    vops += [('fg', i) for i in range(8, 12)]
    vops += [('copy1', j) for j in range(3, 10)]
    vops += [('Sadd', 'h1', 0), ('Sadd', 'h1', 1), ('Sadd', 'h2', 0)]
    vops += [('p', ci) for ci in range(3)]
    vops += [('x', ci) for ci in range(3)]
    vops += [('p', 3), ('p', 4), ('x', 3), ('x', 4)]
    # h1 hist interleaved with the exp-gated S0 adds so vector never idles
    for i in range(6):
        vops += [('histv', 2 * i), ('histv', 2 * i + 1), ('Sadd', 'h0', i)]
    vops += [('p', ci) for ci in range(5, 12)]
    vops += [('x', ci) for ci in range(5, 12)]
    for ci in range(5, 12):
        vops += [('zfh', ci), ('zfd', ci)]
    vops += [('dsnSv', i) for i in range(6)]
    vops += [('histv', i) for i in range(12, len(V_PASSES))]
    # scalar ops
    sops = []
    for q in range(29):
        sops.append(('exp', q))
        if q == 5:
            sops += [('ln', 'h1'), ('rexp', 'h1')]
        if q == 8:
            sops += [('ln', 'h2'), ('rexp', 'h2')]
        if q == 18:
            sops += [('ln', 'h0'), ('rexp', 'h0')]
    sops += [('hists', n) for n in range(len(S_PASSES))]
    sops.append(('lnd',))
    # pool ops
    pops = [('memset',)]

    tidx = {op: i + 1 for i, op in enumerate(tops)}
    vidx = {op: i + 1 for i, op in enumerate(vops)}
    sidx = {op: i + 1 for i, op in enumerate(sops)}
    pidx = {op: i + 1 for i, op in enumerate(pops)}

    from contextlib import ExitStack
    with ExitStack() as es:
        def sb(name, shape, dtype=F32):
            return es.enter_context(nc.sbuf_tensor(name, shape, dtype))

        preds_sb = sb("preds_sb", [49, NCH * 96], BF16)
        uyt_sb = sb("uyt_sb", [49, 192], BF16)
        ux_sb = sb("ux_sb", [96, 384], BF16)
        uxT_sb = sb("uxT_sb", [128, 3 * 96], BF16)
        tf_sb = sb("tf_sb", [128, 3 * 576], BF16)
        t1_sb = sb("t1_sb", [96, NCH * 192], BF16)
        e_sb = sb("e_sb", [128, NCH * 576], BF16)
        s_sb = sb("s_sb", [128, 4 * 576], BF16)
        r_sb = sb("r_sb", [128, 3 * 576], BF16)
        ln_sb = sb("ln_sb", [128, 576])
        fg_sb = sb("fg_sb", [128, 12 * 576], BF16)
        xb_sb = sb("xb_sb", [128, 12 * 576], BF16)
        cst_sb = sb("cst_sb", [128, 4])
        junkv_sb = sb("junkv_sb", [128, 576], BF16)
        junks_sb = sb("junks_sb", [128, 576], BF16)
        acc_sb = sb("acc_sb", [128, ACC_W])

        ps1 = [es.enter_context(nc.psum_tensor(f"ps1{i}", [96, 384], F32)) for i in range(3)]
        ps2 = [es.enter_context(nc.psum_tensor(f"ps2{i}", [128, 384], F32)) for i in range(3)]
        psA = [es.enter_context(nc.psum_tensor(f"psA{i}", [96, 192], F32)) for i in range(2)]

        for i, val in enumerate(BIAS_VALS):
            nc.const_aps.aps[(F32, val)] = cst_sb[:, i: i + 1]

        dmaP = es.enter_context(nc.semaphore("dmaP"))
        dmaU = es.enter_context(nc.semaphore("dmaU"))
        dmaX = es.enter_context(nc.semaphore("dmaX"))
        dmaXT = es.enter_context(nc.semaphore("dmaXT"))
        dmaT = es.enter_context(nc.semaphore("dmaT"))
        dmaC = es.enter_context(nc.semaphore("dmaC"))
        t_sem = es.enter_context(nc.semaphore("t_sem"))
        v_sem = es.enter_context(nc.semaphore("v_sem"))
        s_sem = es.enter_context(nc.semaphore("s_sem"))
        p_sem = es.enter_context(nc.semaphore("p_sem"))
        odma = es.enter_context(nc.semaphore("odma"))

        SEMS = {'t': t_sem, 'v': v_sem, 's': s_sem, 'p': p_sem,
                'P': dmaP, 'U': dmaU, 'X': dmaX, 'XT': dmaXT, 'T': dmaT,
                'C': dmaC}
        IDX = {'t': tidx, 'v': vidx, 's': sidx, 'p': pidx}

        def mk_waiter(eng):
            seen = {}
            def wait(dom, tag=None):
                sem = SEMS[dom]
                n = 16 if tag is None else IDX[dom][tag]
                if seen.get(dom, 0) >= n:
                    return
                seen[dom] = n
                eng.wait_ge(sem, n)
            return wait

        # slice helpers
        def e_ch(c):
            return e_sb[:, 576 * c: 576 * (c + 1)]

        def t1_ch(c):
            return t1_sb[0:96, 192 * c: 192 * (c + 1)]

        def fg_t(ci):
            return fg_sb[:, 576 * ci: 576 * (ci + 1)]

        def xb_t(ci):
            return xb_sb[:, 576 * ci: 576 * (ci + 1)]

        def s_t(h):
            return s_sb[:, S_OFF[h]: S_OFF[h] + 576]

        def r_t(h):
            return r_sb[:, R_OFF[h]: R_OFF[h] + 576]

        def tf_head(ci):
            head = CL[ci][0]
            off = {"h0": 0, "h1": 576, "h2": 1152}[head]
            return tf_sb[:, off: off + 576]

        def acc_col(col, rows=128):
            return acc_sb[0:rows, col: col + 1]

        # exp bank boundary helpers
        def expbank_of_chunk(m):
            return m // 2

        def e_ready_bank(c):
            """exp bank index that completes channel c's tile."""
            return expbank_of_chunk(3 * c + 2)

        with nc.Block() as block:

            @block.sync
            def _(sync):
                sync.dma_start(out=preds_sb[:, :], in_=p_preds[:, :]).then_inc(dmaP, 16)
                sync.dma_start(out=uyt_sb[:, :], in_=p_uyt[:, :]).then_inc(dmaU, 16)
                sync.dma_start(out=ux_sb[:, :], in_=p_ux[:, :]).then_inc(dmaX, 16)
                sync.dma_start(out=uxT_sb[:, :], in_=p_uxT[:, :]).then_inc(dmaXT, 16)
                sync.dma_start(out=tf_sb[:, :], in_=p_tgt[:, :]).then_inc(dmaT, 16)
                sync.dma_start(out=cst_sb[:, :], in_=p_cst[:, :]).then_inc(dmaC, 16)
                sync.wait_ge(v_sem, len(vops))
                sync.wait_ge(s_sem, len(sops))
                sync.wait_ge(p_sem, len(pops))
                sync.dma_start(out=p_acc[:, :], in_=acc_sb[:, :]).then_inc(odma, 16)
                sync.wait_ge(odma, 16)

            @block.tensor
            def _(tensor):
                wait = mk_waiter(tensor)
                for op in tops:
                    if op[0] == 'mm1':
                        c = op[1]
                        if c == 0:
                            wait('P'); wait('U')
                        j = c // 2
                        if c % 2 == 0 and j >= 3:
                            wait('v', ('copy1', j - 3))
                        tensor.matmul(
                            ps1[j % 3][0:96, 192 * (c % 2): 192 * (c % 2) + 192],
                            preds_sb[0:49, 96 * c: 96 * (c + 1)],
                            uyt_sb[0:49, 0:192],
                            start=True, stop=True,
                        ).then_inc(t_sem)
                    elif op[0] == 'mm2':
                        m = op[1]
                        c, k = divmod(m, 3)
                        q = m // 2
                        if m == 0:
                            wait('X')
                        wait('v', ('copy1', c // 2))
                        if q >= 3 and m % 2 == 0:
                            wait('s', ('exp', q - 3))
                        tensor.matmul(
                            ps2[q % 3][0:128, 192 * (m % 2): 192 * (m % 2) + 192],
                            ux_sb[0:96, 128 * k: 128 * (k + 1)],
                            t1_ch(c),
                            start=True, stop=True,
                        ).then_inc(t_sem)
                    else:  # A matmul
                        _, ci, k = op
                        if k == 0:
                            wait('XT')
                            wait('v', ('fg', ci))
                            if ci >= 7:
                                wait('v', ('zfd', ci - 2))
                        tensor.matmul(
                            psA[ci % 2][0:96, 0:192],
                            uxT_sb[0:128, 96 * k: 96 * (k + 1)],
                            fg_sb[:, 576 * ci + 192 * k: 576 * ci + 192 * (k + 1)],
                            start=(k == 0), stop=(k == 2),
                        ).then_inc(t_sem)

            @block.scalar
            def _(scalar):
                wait = mk_waiter(scalar)
                for op in sops:
                    if op[0] == 'exp':
                        q = op[1]
                        w = 384 if q < 28 else 192
                        wait('t', ('mm2', min(2 * q + 1, 56)))
                        scalar.activation(
                            e_sb[:, 384 * q: 384 * q + w],
                            ps2[q % 3][0:128, 0:w], AF.Exp,
                        ).then_inc(s_sem)
                    elif op[0] == 'ln':
                        h = op[1]
                        if h == 'h1':
                            wait('v', ('Sadd', 'h1', 1))
                            scalar.activation(ln_sb[:, :], s_t('h1'), AF.Ln).then_inc(s_sem)
                        elif h == 'h2':
                            wait('v', ('Sadd', 'h2', 0))
                            scalar.activation(ln_sb[:, :], s_t('h2'), AF.Ln).then_inc(s_sem)
                        else:
                            wait('v', ('Sadd', 'h0', 5))
                            wait('p', ('memset',))
                            scalar.activation(
                                ln_sb[:, :], s_t('h0'), AF.Ln,
                                accum_out=acc_col(COL_LNS0),
                            ).then_inc(s_sem)
                    elif op[0] == 'rexp':
                        h = op[1]
                        scalar.activation(r_t(h), ln_sb[:, :], AF.Exp, scale=-1.0).then_inc(s_sem)
                    elif op[0] == 'lnd':
                        wait('v', ('dsnSv', 5))
                        scalar.activation(
                            junks_sb[:, :], s_t('d'), AF.Ln,
                            accum_out=acc_col(COL_LNSD),
                        ).then_inc(s_sem)
                    else:  # hists
                        n = op[1]
                        ci, side, j = S_PASSES[n]
                        t = thr_of(ci)[j]
                        wait('C')
                        wait('v', ('x', ci))
                        scalar.activation(
                            junks_sb[:, :], xb_t(ci), AF.Relu,
                            bias=-t, scale=(1.0 if side == 'F' else -1.0),
                            accum_out=acc_col(col_of(ci, side, j)),
                        ).then_inc(s_sem)

            @block.vector
            def _(vector):
                wait = mk_waiter(vector)
                first_fg = True
                first_acc = True
                for op in vops:
                    if op[0] == 'copy1':
                        j = op[1]
                        w = 384 if j < 9 else 192
                        wait('t', ('mm1', min(2 * j + 1, 18)))
                        vector.tensor_copy(
                            t1_sb[0:96, 384 * j: 384 * j + w],
                            ps1[j % 3][0:96, 0:w],
                        ).then_inc(v_sem)
                    elif op[0] == 'fg':
                        ci = op[1]
                        if first_fg:
                            wait('T')
                            first_fg = False
                        head, c = CL[ci]
                        vector.tensor_scalar(
                            fg_t(ci), tf_head(ci), float(c), 0.0,
                            OP.is_equal, OP.add,
                        ).then_inc(v_sem)
                    elif op[0] == 'Sadd':
                        _, h, i = op
                        if h == 'h1':
                            if i == 0:
                                wait('s', ('exp', e_ready_bank(1)))
                                vector.tensor_add(s_t('h1'), e_ch(0), e_ch(1)).then_inc(v_sem)
                            else:
                                wait('s', ('exp', e_ready_bank(2)))
                                vector.tensor_add(s_t('h1'), s_t('h1'), e_ch(2)).then_inc(v_sem)
                        elif h == 'h2':
                            wait('s', ('exp', e_ready_bank(4)))
                            vector.tensor_add(s_t('h2'), e_ch(3), e_ch(4)).then_inc(v_sem)
                        else:
                            if i == 0:
                                wait('s', ('exp', e_ready_bank(6)))
                                vector.tensor_add(s_t('h0'), e_ch(5), e_ch(6)).then_inc(v_sem)
                            else:
                                wait('s', ('exp', e_ready_bank(6 + i)))
                                vector.tensor_add(s_t('h0'), s_t('h0'), e_ch(6 + i)).then_inc(v_sem)
                    elif op[0] == 'p':
                        ci = op[1]
                        head = CL[ci][0]
                        wait('s', ('rexp', head))
                        ch = chan_of(ci)
                        vector.tensor_mul(e_ch(ch), e_ch(ch), r_t(head)).then_inc(v_sem)
                    elif op[0] == 'x':
                        ci = op[1]
                        vector.tensor_tensor(
                            xb_t(ci), fg_t(ci), e_ch(chan_of(ci)), OP.subtract,
                        ).then_inc(v_sem)
                    elif op[0] == 'histv':
                        n = op[1]
                        ci, side, j = V_PASSES[n]
                        t = thr_of(ci)[j]
                        if first_acc:
                            wait('p', ('memset',))
                            first_acc = False
                        cl = acc_col(col_of(ci, side, j))
                        if side == 'F':
                            vector.tensor_scalar(junkv_sb[:, :], xb_t(ci), t, 0.0,
                                                 OP.max, OP.add, accum_out=cl).then_inc(v_sem)
                        else:
                            vector.tensor_scalar(junkv_sb[:, :], xb_t(ci), -t, 0.0,
                                                 OP.min, OP.add, accum_out=cl).then_inc(v_sem)
                    elif op[0] == 'zfh':
                        ci = op[1]
                        wait('t', ('A', ci, 2))
                        if first_acc:
                            wait('p', ('memset',))
                            first_acc = False
                        vector.scalar_tensor_tensor(
                            junkv_sb[0:96, 0:192], t1_ch(ci), 1.0,
                            psA[ci % 2][0:96, 0:192], OP.mult, OP.mult,
                            accum_out=acc_col(col_zf(ci, 0), rows=96),
                        ).then_inc(v_sem)
                    elif op[0] == 'zfd':
                        ci = op[1]
                        vector.scalar_tensor_tensor(
                            junkv_sb[0:96, 0:192], t1_ch(ci + 7), 1.0,
                            psA[ci % 2][0:96, 0:192], OP.mult, OP.mult,
                            accum_out=acc_col(col_zf(ci, 1), rows=96),
                        ).then_inc(v_sem)
                    else:  # dsnSv
                        i = op[1]
                        if i == 0:
                            wait('s', ('exp', e_ready_bank(13)))
                            vector.tensor_add(s_t('d'), e_ch(12), e_ch(13)).then_inc(v_sem)
                        else:
                            wait('s', ('exp', e_ready_bank(13 + i)))
                            vector.tensor_add(s_t('d'), s_t('d'), e_ch(13 + i)).then_inc(v_sem)

            @block.gpsimd
            def _(gpsimd):
                wait = mk_waiter(gpsimd)
                for op in pops:
                    if op[0] == 'memset':
                        gpsimd.memset(acc_sb[:, :], 0.0).then_inc(p_sem)


    return nc


# ---------------------------------------------------------------- host side --

def _interp_weights():
    s = np.linspace(np.float32(0.0), np.float32(95.0), 384).astype(np.float32)
    i0 = np.clip(np.floor(s).astype(np.int64), 0, 94)
    t = (s - i0).astype(np.float32)
    return i0, t


_CHAN_SRC = ([("preds1", c) for c in range(3)] + [("preds2", c) for c in range(2)]
             + [("preds0", c) for c in range(7)] + [("preds_dsn", c) for c in range(7)])


def _prep_core(inputs, core):
    b, half = core // 2, core % 2
    r0 = half * 192
    cy0 = 0 if half == 0 else 47
    i0, t = _interp_weights()

    uyt = np.zeros((49, 192), np.float32)
    for fy in range(192):
        f = r0 + fy
        uyt[i0[f] - cy0, fy] += np.float32(1.0) - t[f]
        uyt[i0[f] + 1 - cy0, fy] += t[f]

    ux = np.zeros((96, 384), np.float32)
    for X in range(384):
        ux[i0[X], X] += np.float32(1.0) - t[X]
        ux[i0[X] + 1, X] += t[X]
    ux = ux.astype(BF)
    uxT = np.zeros((128, 3 * 96), BF)
    for k in range(3):
        uxT[:, 96 * k: 96 * (k + 1)] = ux[:, 128 * k: 128 * (k + 1)].T

    pa = np.zeros((49, NCH * 96), BF)
    for idx, (key, ch) in enumerate(_CHAN_SRC):
        pa[:, idx * 96: (idx + 1) * 96] = inputs[key][b, ch, cy0: cy0 + 49, :].astype(BF)

    tg = np.zeros((128, 3 * 576), BF)
    for h, key in enumerate(["targets0", "targets1", "targets2"]):
        th = inputs[key][b, r0: r0 + 192, :]
        tg[:, 576 * h: 576 * (h + 1)] = (
            th.reshape(192, 3, 128).transpose(2, 1, 0).reshape(128, 576)
        ).astype(BF)

    cst = np.tile(np.asarray(BIAS_VALS + [0.0], np.float32), (128, 1))
    return {"preds": pa, "uyt": uyt.astype(BF), "ux": ux, "uxT": uxT, "tgt": tg,
            "cst": cst}


def _ncs_core(inputs, core):
    """Exact per-class pixel counts for this shard, from integer targets."""
    b, half = core // 2, core % 2
    r0 = half * 192
    ncs = []
    for ci, (head, c) in enumerate(CL):
        key = {"h1": "targets1", "h2": "targets2", "h0": "targets0"}[head]
        lab = inputs[key][b, r0: r0 + 192, :]
        ncs.append(float((lab == c).sum()))
    return ncs


def _finale(accs, ncs_all):
    lov_total = 0.0
    ce0_num = 0.0
    ced_num = 0.0
    for acc, ncs in zip(accs, ncs_all):
        cs = acc.astype(np.float64).sum(axis=0)
        head_lov = {"h1": [], "h2": [], "h0": []}
        for ci, (head, c) in enumerate(CL):
            thr = thr_of(ci)
            K = len(thr)
            base = 16 * ci
            n_c = ncs[ci]
            TF, TB = [], []
            for j, t in enumerate(thr):
                cF = cs[base + 6 + j]
                cB = cs[base + j]
                if (ci, 'F', j) in SCALAR_SET:
                    TF.append(cF)
                else:
                    TF.append(cF - N_PIX * t)
                if (ci, 'B', j) in SCALAR_SET:
                    TB.append(cB)
                else:
                    TB.append(-cB - N_PIX * t)
            TF.append(0.0)
            TB.append(0.0)
            if n_c < 0.5:
                continue
            ts_ext = list(thr) + [1.0]
            L = 0.0
            for j in range(K):
                IF = TF[j] - TF[j + 1]
                IB = TB[j] - TB[j + 1]
                d = ts_ext[j + 1] - ts_ext[j]
                L += (IF + IB) / (n_c + IB / d)
            head_lov[head].append(L)
        for head, w in (("h0", 1.0), ("h1", 0.4), ("h2", 0.4)):
            vals = head_lov[head]
            lov_total += w * (sum(vals) / max(len(vals), 1))
        ce0_num += cs[COL_LNS0] - sum(cs[16 * ci + 13] for ci in range(5, 12))
        ced_num += cs[COL_LNSD] - sum(cs[16 * ci + 14] for ci in range(5, 12))
    return ce0_num / P_GLOBAL + 0.4 * (ced_num / P_GLOBAL) + lov_total / 8.0


_NC_CACHE = None


def kernel(**inputs):
    global _NC_CACHE
    inputs = {k: np.asarray(v) for k, v in inputs.items()}
    if _NC_CACHE is None:
        _NC_CACHE = build_kernel()
    nc = _NC_CACHE
    in_maps = [_prep_core(inputs, core) for core in range(8)]
    res = run_bass_kernel_spmd(nc, in_maps, core_ids=list(range(8)))
    accs = [np.asarray(res.results[c]["acc"], dtype=np.float32) for c in range(8)]
    ncs_all = [_ncs_core(inputs, c) for c in range(8)]
    loss = _finale(accs, ncs_all)
    return np.asarray(loss, dtype=np.float32)


# revision 14
# speedup vs baseline: 2.6643x; 1.0877x over previous
"""Trainium2 distributed kernel for ABRLovaszCELoss (8 NeuronCores).

Strategy (v4)
-------------
Data-parallel over (batch, row-half): core i handles batch b=i//2, fine rows
[192*(i%2), 192*(i%2)+192) of the 384x384 target grid (73728 pixels/core).

Per core, fully on-device (all-bf16 datapath):
- bilinear align_corners upsample 96->384 of all 19 logit channels
  (order head1:3, head2:2, head0:7, dsn:7) as two bf16 PE matmuls per
  channel; pixel layout [128 part = X%128, 576 free = 192*(X//128) + fy].
- stage-1 PSUM pairs copied to SBUF bf16 by vector; stage-2 pairs exp'd
  384-wide on scalar into one contiguous e_all tile.
- softmax: per-head S sums on vector (dsn S on pool), r = exp(-ln S) on
  scalar; the head0 Ln pass also accumulates CE's sum(ln S); p = e*r
  in-place; x = fg - p; fg masks via fast tensor_scalar is_equal.
- CE's sum(z*[tgt==c]) via linearity of the interpolation:
  <t1_c, A_c> where A_c = ux^T-adjoint of the fg mask (3 accumulating PE
  matmuls per head0 class), consumed by tiny [96,192] stt accumulations.
  n_c is computed exactly on the host from the integer targets.
- Lovasz-Softmax per shard via exact relu tail-integrals (no sort):
  TF_j = sum relu(x - t_j), TB_j = sum relu(-x - t_j) at bf16-exact
  thresholds; per-segment integrals by differencing on the host, and
  L_c = sum_j (IF_j + IB_j) / (n_c + IB_j/dt_j).
  Histogram passes split across vector (tensor_scalar add-reduce accum),
  scalar (Relu+accum), and pool (vector-prepped relu tiles + XYZWC
  tensor_reduce).
- the [128,256] accumulator tile is DMA'd out per core; the final
  per-class differencing/reciprocal algebra and the 8-shard reduction
  happen on the host during gather/unshard (exact fp64).
"""

import numpy as np
import ml_dtypes

import concourse.bass as bass
import concourse.mybir as mybir
from concourse.bass_utils import run_bass_kernel_spmd

F32 = mybir.dt.float32
BF16 = mybir.dt.bfloat16
AF = mybir.ActivationFunctionType
OP = mybir.AluOpType
AX = mybir.AxisListType
BF = ml_dtypes.bfloat16

NCH = 19
N_PIX = 73728
P_GLOBAL = 4 * 384 * 384

# channel order: head1 (3), head2 (2), head0 (7), dsn (7)
THR12 = (0.0, 0.34375, 0.671875)   # bf16-exact ~ j/3
THR0 = (0.0, 0.5)

# lovasz classes in "CL" order
CL = ([("h1", c) for c in range(3)] + [("h2", c) for c in range(2)]
      + [("h0", c) for c in range(7)])
HEAD_CH0 = {"h1": 0, "h2": 3, "h0": 5, "d": 12}
S_OFF = {"h1": 0, "h2": 576, "h0": 1152, "d": 1728}
R_OFF = {"h1": 0, "h2": 576, "h0": 1152}


def chan_of(ci):
    head, c = CL[ci]
    return HEAD_CH0[head] + c


def thr_of(ci):
    return THR12 if ci < 5 else THR0


# histogram pass assignment (side 'B' = TB via min/relu(-x-t), 'F' = TF)
# vector: h1+h2+h0_5+h0_6 (tensor_scalar add-reduce); scalar: h0_0..h0_4
V_PASSES = ([(ci, s, j) for ci in range(5) for s in 'BF' for j in range(3)]
            + [(ci, s, j) for ci in (10, 11) for s in 'BF' for j in range(2)])
S_PASSES = [(ci, s, j) for ci in range(5, 10) for s in 'BF' for j in range(2)]
SCALAR_SET = set(S_PASSES)

ACC_W = 256
COL_LNS0 = 192
COL_LNSD = 193


def col_of(ci, side, j):
    return 16 * ci + (j if side == 'B' else 6 + j)


def col_zf(ci, which):   # which: 0 = head0, 1 = dsn
    return 16 * ci + 13 + which


N_JR = 6   # jr ring slots


BIAS_VALS = sorted({-t for t in THR0[1:]} | {-t for t in THR12[1:]})


def build_kernel():
    nc = bass.Bass()

    p_cst = nc.declare_dram_parameter("cst", [128, 4], F32, isOutput=False)
    p_preds = nc.declare_dram_parameter("preds", [49, NCH * 96], BF16, isOutput=False)
    p_uyt = nc.declare_dram_parameter("uyt", [49, 192], BF16, isOutput=False)
    p_ux = nc.declare_dram_parameter("ux", [96, 384], BF16, isOutput=False)
    p_uxT = nc.declare_dram_parameter("uxT", [128, 3 * 96], BF16, isOutput=False)
    p_tgt = nc.declare_dram_parameter("tgt", [128, 3 * 576], BF16, isOutput=False)
    p_acc = nc.declare_dram_parameter("acc", [128, ACC_W], F32, isOutput=True)

    # ---------------- static program-order op lists (for cross-engine idx) --
    # tensor ops
    # A(ci) is inserted after stage-2 chunk 31+2*ci (i.e. after ps2 pair
    # 15+ci) so its wait on V's zfg consumption can never stall stage-2
    # pairs that gate the exps the V-side softmax chain needs.
    # mm2 chunks 0..5 (ps2 pairs 0..2, no exp wait) are emitted early so the
    # scalar exp stream starts while the remaining mm1s are still flowing;
    # all exp-gated mm2 chunks come after every mm1 (V's stage-1 copies must
    # never transitively depend on the scalar stream).
    tops = []
    A_after = {31 + 2 * ci: ci for ci in range(5, 12)}
    for c in range(6):
        tops.append(('mm1', c))
    for m in range(6):
        tops.append(('mm2', m))
    for c in range(6, NCH):
        tops.append(('mm1', c))
    for m in range(6, 57):
        tops.append(('mm2', m))
        if m in A_after:
            for k in range(3):
                tops.append(('A', A_after[m], k))
    # vector ops
    vops = []
    vops.append(('copy1', 0))
    vops += [('fg', i) for i in range(4)]
    vops.append(('copy1', 1))
    vops += [('fg', i) for i in range(4, 8)]
    vops.append(('copy1', 2))
    vops += [('fg', i) for i in range(8, 12)]
    vops += [('copy1', j) for j in range(3, 10)]
    vops += [('Sadd', 'h1', 0), ('Sadd', 'h1', 1), ('Sadd', 'h2', 0)]
    vops += [('p', ci) for ci in range(3)]
    vops += [('x', ci) for ci in range(3)]
    vops += [('p', 3), ('p', 4), ('x', 3), ('x', 4)]
    # h1 hist interleaved with the exp-gated S0 adds so vector never idles
    for i in range(6):
        vops += [('histv', 3 * i), ('histv', 3 * i + 1), ('histv', 3 * i + 2),
                 ('Sadd', 'h0', i)]
    vops += [('p', ci) for ci in range(5, 12)]
    vops += [('x', ci) for ci in range(5, 12)]
    for ci in range(5, 12):
        vops += [('zfh', ci), ('zfd', ci)]
    vops += [('dsnSv', i) for i in range(6)]
    vops += [('histv', i) for i in range(18, len(V_PASSES))]
    # scalar ops
    sops = [('warm',)]
    for q in range(29):
        sops.append(('exp', q))
        if q == 5:
            sops += [('ln', 'h1'), ('rexp', 'h1')]
        if q == 8:
            sops += [('ln', 'h2'), ('rexp', 'h2')]
        if q == 18:
            sops += [('ln', 'h0'), ('rexp', 'h0')]
    sops += [('hists', n) for n in range(len(S_PASSES))]
    sops.append(('lnd',))
    # pool ops
    pops = [('memset',)]

    tidx = {op: i + 1 for i, op in enumerate(tops)}
    vidx = {op: i + 1 for i, op in enumerate(vops)}
    sidx = {op: i + 1 for i, op in enumerate(sops)}
    pidx = {op: i + 1 for i, op in enumerate(pops)}

    from contextlib import ExitStack
    with ExitStack() as es:
        def sb(name, shape, dtype=F32):
            return es.enter_context(nc.sbuf_tensor(name, shape, dtype))

        preds_sb = sb("preds_sb", [49, NCH * 96], BF16)
        uyt_sb = sb("uyt_sb", [49, 192], BF16)
        ux_sb = sb("ux_sb", [96, 384], BF16)
        uxT_sb = sb("uxT_sb", [128, 3 * 96], BF16)
        tf_sb = sb("tf_sb", [128, 3 * 576], BF16)
        t1_sb = sb("t1_sb", [96, NCH * 192], BF16)
        e_sb = sb("e_sb", [128, NCH * 576], BF16)
        s_sb = sb("s_sb", [128, 4 * 576], BF16)
        r_sb = sb("r_sb", [128, 3 * 576], BF16)
        ln_sb = sb("ln_sb", [128, 576])
        fg_sb = sb("fg_sb", [128, 12 * 576], BF16)
        xb_sb = sb("xb_sb", [128, 12 * 576], BF16)
        cst_sb = sb("cst_sb", [128, 4])
        junkv_sb = sb("junkv_sb", [128, 576], BF16)
        junks_sb = sb("junks_sb", [128, 576], BF16)
        acc_sb = sb("acc_sb", [128, ACC_W])

        ps1 = [es.enter_context(nc.psum_tensor(f"ps1{i}", [96, 384], F32)) for i in range(3)]
        ps2 = [es.enter_context(nc.psum_tensor(f"ps2{i}", [128, 384], F32)) for i in range(3)]
        psA = [es.enter_context(nc.psum_tensor(f"psA{i}", [96, 192], F32)) for i in range(2)]

        for i, val in enumerate(BIAS_VALS):
            nc.const_aps.aps[(F32, val)] = cst_sb[:, i: i + 1]

        dmaP = es.enter_context(nc.semaphore("dmaP"))
        dmaU = es.enter_context(nc.semaphore("dmaU"))
        dmaX = es.enter_context(nc.semaphore("dmaX"))
        dmaXT = es.enter_context(nc.semaphore("dmaXT"))
        dmaT = es.enter_context(nc.semaphore("dmaT"))
        dmaC = es.enter_context(nc.semaphore("dmaC"))
        t_sem = es.enter_context(nc.semaphore("t_sem"))
        v_sem = es.enter_context(nc.semaphore("v_sem"))
        s_sem = es.enter_context(nc.semaphore("s_sem"))
        p_sem = es.enter_context(nc.semaphore("p_sem"))
        odma = es.enter_context(nc.semaphore("odma"))

        SEMS = {'t': t_sem, 'v': v_sem, 's': s_sem, 'p': p_sem,
                'P': dmaP, 'U': dmaU, 'X': dmaX, 'XT': dmaXT, 'T': dmaT,
                'C': dmaC}
        IDX = {'t': tidx, 'v': vidx, 's': sidx, 'p': pidx}

        def mk_waiter(eng):
            seen = {}
            def wait(dom, tag=None):
                sem = SEMS[dom]
                n = 16 if tag is None else IDX[dom][tag]
                if seen.get(dom, 0) >= n:
                    return
                seen[dom] = n
                eng.wait_ge(sem, n)
            return wait

        # slice helpers
        def e_ch(c):
            return e_sb[:, 576 * c: 576 * (c + 1)]

        def t1_ch(c):
            return t1_sb[0:96, 192 * c: 192 * (c + 1)]

        def fg_t(ci):
            return fg_sb[:, 576 * ci: 576 * (ci + 1)]

        def xb_t(ci):
            return xb_sb[:, 576 * ci: 576 * (ci + 1)]

        def s_t(h):
            return s_sb[:, S_OFF[h]: S_OFF[h] + 576]

        def r_t(h):
            return r_sb[:, R_OFF[h]: R_OFF[h] + 576]

        def tf_head(ci):
            head = CL[ci][0]
            off = {"h0": 0, "h1": 576, "h2": 1152}[head]
            return tf_sb[:, off: off + 576]

        def acc_col(col, rows=128):
            return acc_sb[0:rows, col: col + 1]

        # exp bank boundary helpers
        def expbank_of_chunk(m):
            return m // 2

        def e_ready_bank(c):
            """exp bank index that completes channel c's tile."""
            return expbank_of_chunk(3 * c + 2)

        with nc.Block() as block:

            @block.sync
            def _(sync):
                sync.dma_start(out=preds_sb[:, :], in_=p_preds[:, :]).then_inc(dmaP, 16)
                sync.dma_start(out=uyt_sb[:, :], in_=p_uyt[:, :]).then_inc(dmaU, 16)
                sync.dma_start(out=tf_sb[:, :], in_=p_tgt[:, :]).then_inc(dmaT, 16)
                sync.dma_start(out=ux_sb[:, :], in_=p_ux[:, :]).then_inc(dmaX, 16)
                sync.dma_start(out=uxT_sb[:, :], in_=p_uxT[:, :]).then_inc(dmaXT, 16)
                sync.dma_start(out=cst_sb[:, :], in_=p_cst[:, :]).then_inc(dmaC, 16)
                sync.wait_ge(v_sem, len(vops))
                sync.wait_ge(s_sem, len(sops))
                sync.wait_ge(p_sem, len(pops))
                sync.dma_start(out=p_acc[:, :], in_=acc_sb[:, :]).then_inc(odma, 16)
                sync.wait_ge(odma, 16)

            @block.tensor
            def _(tensor):
                wait = mk_waiter(tensor)
                for op in tops:
                    if op[0] == 'mm1':
                        c = op[1]
                        if c == 0:
                            wait('P'); wait('U')
                        j = c // 2
                        if c % 2 == 0 and j >= 3:
                            wait('v', ('copy1', j - 3))
                        tensor.matmul(
                            ps1[j % 3][0:96, 192 * (c % 2): 192 * (c % 2) + 192],
                            preds_sb[0:49, 96 * c: 96 * (c + 1)],
                            uyt_sb[0:49, 0:192],
                            start=True, stop=True,
                        ).then_inc(t_sem)
                    elif op[0] == 'mm2':
                        m = op[1]
                        c, k = divmod(m, 3)
                        q = m // 2
                        if m == 0:
                            wait('X')
                        wait('v', ('copy1', c // 2))
                        if q >= 3 and m % 2 == 0:
                            wait('s', ('exp', q - 3))
                        tensor.matmul(
                            ps2[q % 3][0:128, 192 * (m % 2): 192 * (m % 2) + 192],
                            ux_sb[0:96, 128 * k: 128 * (k + 1)],
                            t1_ch(c),
                            start=True, stop=True,
                        ).then_inc(t_sem)
                    else:  # A matmul
                        _, ci, k = op
                        if k == 0:
                            wait('XT')
                            wait('v', ('fg', ci))
                            if ci >= 7:
                                wait('v', ('zfd', ci - 2))
                        tensor.matmul(
                            psA[ci % 2][0:96, 0:192],
                            uxT_sb[0:128, 96 * k: 96 * (k + 1)],
                            fg_sb[:, 576 * ci + 192 * k: 576 * ci + 192 * (k + 1)],
                            start=(k == 0), stop=(k == 2),
                        ).then_inc(t_sem)

            @block.scalar
            def _(scalar):
                wait = mk_waiter(scalar)
                for op in sops:
                    if op[0] == 'warm':
                        # touch the Exp/Ln act table so the 1.3us table load
                        # happens during DMA startup, off the critical path
                        scalar.activation(junks_sb[0:1, 0:1], junks_sb[0:1, 0:1],
                                          AF.Exp).then_inc(s_sem)
                    elif op[0] == 'exp':
                        q = op[1]
                        w = 384 if q < 28 else 192
                        wait('t', ('mm2', min(2 * q + 1, 56)))
                        scalar.activation(
                            e_sb[:, 384 * q: 384 * q + w],
                            ps2[q % 3][0:128, 0:w], AF.Exp,
                        ).then_inc(s_sem)
                    elif op[0] == 'ln':
                        h = op[1]
                        if h == 'h1':
                            wait('v', ('Sadd', 'h1', 1))
                            scalar.activation(ln_sb[:, :], s_t('h1'), AF.Ln).then_inc(s_sem)
                        elif h == 'h2':
                            wait('v', ('Sadd', 'h2', 0))
                            scalar.activation(ln_sb[:, :], s_t('h2'), AF.Ln).then_inc(s_sem)
                        else:
                            wait('v', ('Sadd', 'h0', 5))
                            wait('p', ('memset',))
                            scalar.activation(
                                ln_sb[:, :], s_t('h0'), AF.Ln,
                                accum_out=acc_col(COL_LNS0),
                            ).then_inc(s_sem)
                    elif op[0] == 'rexp':
                        h = op[1]
                        scalar.activation(r_t(h), ln_sb[:, :], AF.Exp, scale=-1.0).then_inc(s_sem)
                    elif op[0] == 'lnd':
                        wait('v', ('dsnSv', 5))
                        scalar.activation(
                            junks_sb[:, :], s_t('d'), AF.Ln,
                            accum_out=acc_col(COL_LNSD),
                        ).then_inc(s_sem)
                    else:  # hists
                        n = op[1]
                        ci, side, j = S_PASSES[n]
                        t = thr_of(ci)[j]
                        wait('C')
                        wait('v', ('x', ci))
                        scalar.activation(
                            junks_sb[:, :], xb_t(ci), AF.Relu,
                            bias=-t, scale=(1.0 if side == 'F' else -1.0),
                            accum_out=acc_col(col_of(ci, side, j)),
                        ).then_inc(s_sem)

            @block.vector
            def _(vector):
                wait = mk_waiter(vector)
                first_fg = True
                first_acc = True
                for op in vops:
                    if op[0] == 'copy1':
                        j = op[1]
                        w = 384 if j < 9 else 192
                        wait('t', ('mm1', min(2 * j + 1, 18)))
                        vector.tensor_copy(
                            t1_sb[0:96, 384 * j: 384 * j + w],
                            ps1[j % 3][0:96, 0:w],
                        ).then_inc(v_sem)
                    elif op[0] == 'fg':
                        ci = op[1]
                        if first_fg:
                            wait('T')
                            first_fg = False
                        head, c = CL[ci]
                        vector.tensor_scalar(
                            fg_t(ci), tf_head(ci), float(c), 0.0,
                            OP.is_equal, OP.add,
                        ).then_inc(v_sem)
                    elif op[0] == 'Sadd':
                        _, h, i = op
                        if h == 'h1':
                            if i == 0:
                                wait('s', ('exp', e_ready_bank(1)))
                                vector.tensor_add(s_t('h1'), e_ch(0), e_ch(1)).then_inc(v_sem)
                            else:
                                wait('s', ('exp', e_ready_bank(2)))
                                vector.tensor_add(s_t('h1'), s_t('h1'), e_ch(2)).then_inc(v_sem)
                        elif h == 'h2':
                            wait('s', ('exp', e_ready_bank(4)))
                            vector.tensor_add(s_t('h2'), e_ch(3), e_ch(4)).then_inc(v_sem)
                        else:
                            if i == 0:
                                wait('s', ('exp', e_ready_bank(6)))
                                vector.tensor_add(s_t('h0'), e_ch(5), e_ch(6)).then_inc(v_sem)
                            else:
                                wait('s', ('exp', e_ready_bank(6 + i)))
                                vector.tensor_add(s_t('h0'), s_t('h0'), e_ch(6 + i)).then_inc(v_sem)
                    elif op[0] == 'p':
                        ci = op[1]
                        head = CL[ci][0]
                        wait('s', ('rexp', head))
                        ch = chan_of(ci)
                        vector.tensor_mul(e_ch(ch), e_ch(ch), r_t(head)).then_inc(v_sem)
                    elif op[0] == 'x':
                        ci = op[1]
                        vector.tensor_tensor(
                            xb_t(ci), fg_t(ci), e_ch(chan_of(ci)), OP.subtract,
                        ).then_inc(v_sem)
                    elif op[0] == 'histv':
                        n = op[1]
                        ci, side, j = V_PASSES[n]
                        t = thr_of(ci)[j]
                        if first_acc:
                            wait('p', ('memset',))
                            first_acc = False
                        cl = acc_col(col_of(ci, side, j))
                        if side == 'F':
                            vector.tensor_scalar(junkv_sb[:, :], xb_t(ci), t, 0.0,
                                                 OP.max, OP.add, accum_out=cl).then_inc(v_sem)
                        else:
                            vector.tensor_scalar(junkv_sb[:, :], xb_t(ci), -t, 0.0,
                                                 OP.min, OP.add, accum_out=cl).then_inc(v_sem)
                    elif op[0] == 'zfh':
                        ci = op[1]
                        wait('t', ('A', ci, 2))
                        if first_acc:
                            wait('p', ('memset',))
                            first_acc = False
                        vector.scalar_tensor_tensor(
                            junkv_sb[0:96, 0:192], t1_ch(ci), 1.0,
                            psA[ci % 2][0:96, 0:192], OP.mult, OP.mult,
                            accum_out=acc_col(col_zf(ci, 0), rows=96),
                        ).then_inc(v_sem)
                    elif op[0] == 'zfd':
                        ci = op[1]
                        vector.scalar_tensor_tensor(
                            junkv_sb[0:96, 0:192], t1_ch(ci + 7), 1.0,
                            psA[ci % 2][0:96, 0:192], OP.mult, OP.mult,
                            accum_out=acc_col(col_zf(ci, 1), rows=96),
                        ).then_inc(v_sem)
                    else:  # dsnSv
                        i = op[1]
                        if i == 0:
                            wait('s', ('exp', e_ready_bank(13)))
                            vector.tensor_add(s_t('d'), e_ch(12), e_ch(13)).then_inc(v_sem)
                        else:
                            wait('s', ('exp', e_ready_bank(13 + i)))
                            vector.tensor_add(s_t('d'), s_t('d'), e_ch(13 + i)).then_inc(v_sem)

            @block.gpsimd
            def _(gpsimd):
                wait = mk_waiter(gpsimd)
                for op in pops:
                    if op[0] == 'memset':
                        gpsimd.memset(acc_sb[:, :], 0.0).then_inc(p_sem)


    return nc


# ---------------------------------------------------------------- host side --

def _interp_weights():
    s = np.linspace(np.float32(0.0), np.float32(95.0), 384).astype(np.float32)
    i0 = np.clip(np.floor(s).astype(np.int64), 0, 94)
    t = (s - i0).astype(np.float32)
    return i0, t


_CHAN_SRC = ([("preds1", c) for c in range(3)] + [("preds2", c) for c in range(2)]
             + [("preds0", c) for c in range(7)] + [("preds_dsn", c) for c in range(7)])


def _prep_core(inputs, core):
    b, half = core // 2, core % 2
    r0 = half * 192
    cy0 = 0 if half == 0 else 47
    i0, t = _interp_weights()

    uyt = np.zeros((49, 192), np.float32)
    for fy in range(192):
        f = r0 + fy
        uyt[i0[f] - cy0, fy] += np.float32(1.0) - t[f]
        uyt[i0[f] + 1 - cy0, fy] += t[f]

    ux = np.zeros((96, 384), np.float32)
    for X in range(384):
        ux[i0[X], X] += np.float32(1.0) - t[X]
        ux[i0[X] + 1, X] += t[X]
    ux = ux.astype(BF)
    uxT = np.zeros((128, 3 * 96), BF)
    for k in range(3):
        uxT[:, 96 * k: 96 * (k + 1)] = ux[:, 128 * k: 128 * (k + 1)].T

    pa = np.zeros((49, NCH * 96), BF)
    for idx, (key, ch) in enumerate(_CHAN_SRC):
        pa[:, idx * 96: (idx + 1) * 96] = inputs[key][b, ch, cy0: cy0 + 49, :].astype(BF)

    tg = np.zeros((128, 3 * 576), BF)
    for h, key in enumerate(["targets0", "targets1", "targets2"]):
        th = inputs[key][b, r0: r0 + 192, :]
        tg[:, 576 * h: 576 * (h + 1)] = (
            th.reshape(192, 3, 128).transpose(2, 1, 0).reshape(128, 576)
        ).astype(BF)

    cst = np.tile(np.asarray(BIAS_VALS + [0.0], np.float32), (128, 1))
    return {"preds": pa, "uyt": uyt.astype(BF), "ux": ux, "uxT": uxT, "tgt": tg,
            "cst": cst}


def _ncs_core(inputs, core):
    """Exact per-class pixel counts for this shard, from integer targets."""
    b, half = core // 2, core % 2
    r0 = half * 192
    ncs = []
    for ci, (head, c) in enumerate(CL):
        key = {"h1": "targets1", "h2": "targets2", "h0": "targets0"}[head]
        lab = inputs[key][b, r0: r0 + 192, :]
        ncs.append(float((lab == c).sum()))
    return ncs


def _finale(accs, ncs_all):
    lov_total = 0.0
    ce0_num = 0.0
    ced_num = 0.0
    for acc, ncs in zip(accs, ncs_all):
        cs = acc.astype(np.float64).sum(axis=0)
        head_lov = {"h1": [], "h2": [], "h0": []}
        for ci, (head, c) in enumerate(CL):
            thr = thr_of(ci)
            K = len(thr)
            base = 16 * ci
            n_c = ncs[ci]
            TF, TB = [], []
            for j, t in enumerate(thr):
                cF = cs[base + 6 + j]
                cB = cs[base + j]
                if (ci, 'F', j) in SCALAR_SET:
                    TF.append(cF)
                else:
                    TF.append(cF - N_PIX * t)
                if (ci, 'B', j) in SCALAR_SET:
                    TB.append(cB)
                else:
                    TB.append(-cB - N_PIX * t)
            TF.append(0.0)
            TB.append(0.0)
            if n_c < 0.5:
                continue
            ts_ext = list(thr) + [1.0]
            L = 0.0
            for j in range(K):
                IF = TF[j] - TF[j + 1]
                IB = TB[j] - TB[j + 1]
                d = ts_ext[j + 1] - ts_ext[j]
                L += (IF + IB) / (n_c + IB / d)
            head_lov[head].append(L)
        for head, w in (("h0", 1.0), ("h1", 0.4), ("h2", 0.4)):
            vals = head_lov[head]
            lov_total += w * (sum(vals) / max(len(vals), 1))
        ce0_num += cs[COL_LNS0] - sum(cs[16 * ci + 13] for ci in range(5, 12))
        ced_num += cs[COL_LNSD] - sum(cs[16 * ci + 14] for ci in range(5, 12))
    return ce0_num / P_GLOBAL + 0.4 * (ced_num / P_GLOBAL) + lov_total / 8.0


_NC_CACHE = None


def kernel(**inputs):
    global _NC_CACHE
    inputs = {k: np.asarray(v) for k, v in inputs.items()}
    if _NC_CACHE is None:
        _NC_CACHE = build_kernel()
    nc = _NC_CACHE
    in_maps = [_prep_core(inputs, core) for core in range(8)]
    res = run_bass_kernel_spmd(nc, in_maps, core_ids=list(range(8)))
    accs = [np.asarray(res.results[c]["acc"], dtype=np.float32) for c in range(8)]
    ncs_all = [_ncs_core(inputs, c) for c in range(8)]
    loss = _finale(accs, ncs_all)
    return np.asarray(loss, dtype=np.float32)


# revision 15
# speedup vs baseline: 2.7971x; 1.0498x over previous
"""Trainium2 distributed kernel for ABRLovaszCELoss (8 NeuronCores).

Strategy (v4)
-------------
Data-parallel over (batch, row-half): core i handles batch b=i//2, fine rows
[192*(i%2), 192*(i%2)+192) of the 384x384 target grid (73728 pixels/core).

Per core, fully on-device (all-bf16 datapath):
- bilinear align_corners upsample 96->384 of all 19 logit channels
  (order head1:3, head2:2, head0:7, dsn:7) as two bf16 PE matmuls per
  channel; pixel layout [128 part = X%128, 576 free = 192*(X//128) + fy].
- stage-1 PSUM pairs copied to SBUF bf16 by vector; stage-2 pairs exp'd
  384-wide on scalar into one contiguous e_all tile.
- softmax: per-head S sums on vector (dsn S on pool), r = exp(-ln S) on
  scalar; the head0 Ln pass also accumulates CE's sum(ln S); p = e*r
  in-place; x = fg - p; fg masks via fast tensor_scalar is_equal.
- CE's sum(z*[tgt==c]) via linearity of the interpolation:
  <t1_c, A_c> where A_c = ux^T-adjoint of the fg mask (3 accumulating PE
  matmuls per head0 class), consumed by tiny [96,192] stt accumulations.
  n_c is computed exactly on the host from the integer targets.
- Lovasz-Softmax per shard via exact relu tail-integrals (no sort):
  TF_j = sum relu(x - t_j), TB_j = sum relu(-x - t_j) at bf16-exact
  thresholds; per-segment integrals by differencing on the host, and
  L_c = sum_j (IF_j + IB_j) / (n_c + IB_j/dt_j).
  Histogram passes split across vector (tensor_scalar add-reduce accum),
  scalar (Relu+accum), and pool (vector-prepped relu tiles + XYZWC
  tensor_reduce).
- the [128,256] accumulator tile is DMA'd out per core; the final
  per-class differencing/reciprocal algebra and the 8-shard reduction
  happen on the host during gather/unshard (exact fp64).
"""

import numpy as np
import ml_dtypes

import concourse.bass as bass
import concourse.mybir as mybir
from concourse.bass_utils import run_bass_kernel_spmd

F32 = mybir.dt.float32
BF16 = mybir.dt.bfloat16
AF = mybir.ActivationFunctionType
OP = mybir.AluOpType
AX = mybir.AxisListType
BF = ml_dtypes.bfloat16

NCH = 19
N_PIX = 73728
P_GLOBAL = 4 * 384 * 384

# channel order: head1 (3), head2 (2), head0 (7), dsn (7)
THR12 = (0.0, 0.34375, 0.671875)   # bf16-exact ~ j/3
THR0 = (0.0, 0.5)

# lovasz classes in "CL" order
CL = ([("h1", c) for c in range(3)] + [("h2", c) for c in range(2)]
      + [("h0", c) for c in range(7)])
HEAD_CH0 = {"h1": 0, "h2": 3, "h0": 5, "d": 12}
S_OFF = {"h1": 0, "h2": 576, "h0": 1152, "d": 1728}
R_OFF = {"h1": 0, "h2": 576, "h0": 1152}


def chan_of(ci):
    head, c = CL[ci]
    return HEAD_CH0[head] + c


def thr_of(ci):
    return THR12 if ci < 5 else THR0


# histogram pass assignment (side 'B' = TB via min/relu(-x-t), 'F' = TF)
# vector: h1+h2 (tensor_scalar add-reduce); scalar: all h0 classes
V_PASSES = [(ci, s, j) for ci in range(5) for s in 'BF' for j in range(3)]
S_PASSES = [(ci, s, j) for ci in range(5, 12) for s in 'BF' for j in range(2)]
SCALAR_SET = set(S_PASSES)

ACC_W = 256
COL_LNS0 = 192
COL_LNSD = 193


def col_of(ci, side, j):
    return 16 * ci + (j if side == 'B' else 6 + j)


def col_zf(ci, which):   # which: 0 = head0, 1 = dsn
    return 16 * ci + 13 + which


N_JR = 6   # jr ring slots


BIAS_VALS = sorted({-t for t in THR0[1:]} | {-t for t in THR12[1:]})


def build_kernel():
    nc = bass.Bass()

    p_cst = nc.declare_dram_parameter("cst", [128, 4], F32, isOutput=False)
    p_preds = nc.declare_dram_parameter("preds", [49, NCH * 96], BF16, isOutput=False)
    p_uyt = nc.declare_dram_parameter("uyt", [49, 192], BF16, isOutput=False)
    p_ux = nc.declare_dram_parameter("ux", [96, 384], BF16, isOutput=False)
    p_uxT = nc.declare_dram_parameter("uxT", [128, 3 * 96], BF16, isOutput=False)
    p_tgt = nc.declare_dram_parameter("tgt", [128, 3 * 576], BF16, isOutput=False)
    p_acc = nc.declare_dram_parameter("acc", [128, ACC_W], F32, isOutput=True)

    # ---------------- static program-order op lists (for cross-engine idx) --
    # tensor ops
    # A(ci) is inserted after stage-2 chunk 31+2*ci (i.e. after ps2 pair
    # 15+ci) so its wait on V's zfg consumption can never stall stage-2
    # pairs that gate the exps the V-side softmax chain needs.
    # mm2 chunks 0..5 (ps2 pairs 0..2, no exp wait) are emitted early so the
    # scalar exp stream starts while the remaining mm1s are still flowing;
    # all exp-gated mm2 chunks come after every mm1 (V's stage-1 copies must
    # never transitively depend on the scalar stream).
    tops = []
    A_after = {31 + 2 * ci: ci for ci in range(5, 12)}
    for c in range(6):
        tops.append(('mm1', c))
    for m in range(6):
        tops.append(('mm2', m))
    for c in range(6, NCH):
        tops.append(('mm1', c))
    for m in range(6, 57):
        tops.append(('mm2', m))
        if m in A_after:
            for k in range(3):
                tops.append(('A', A_after[m], k))
    # vector ops: copies first (paces the mm1/mm2 streams), then masks,
    # then the softmax chains; h0's S adds go as early as possible since
    # scalar's ln/rexp (and so p0/x0 and the scalar hist tail) gate on them.
    vops = [('copy1', j) for j in range(10)]
    vops += [('fg', i) for i in range(12)]
    vops += [('Sadd', 'h1', 0), ('Sadd', 'h1', 1), ('Sadd', 'h2', 0)]
    vops += [('p', ci) for ci in range(3)]
    vops += [('x', ci) for ci in range(3)]
    vops += [('p', 3), ('p', 4), ('x', 3), ('x', 4)]
    vops += [('Sadd', 'h0', i) for i in range(6)]
    vops += [('histv', 0), ('histv', 1)]   # filler while rexp(h0) lands
    vops += [('p', ci) for ci in range(5, 12)]
    vops += [('x', ci) for ci in range(5, 12)]
    for ci in range(5, 12):
        vops += [('zfh', ci), ('zfd', ci)]
    vops += [('dsnSv', i) for i in range(6)]
    vops += [('histv', i) for i in range(2, len(V_PASSES))]
    # scalar ops
    sops = [('warm',)]
    for q in range(29):
        sops.append(('exp', q))
        if q == 5:
            sops += [('ln', 'h1'), ('rexp', 'h1')]
        if q == 8:
            sops += [('ln', 'h2'), ('rexp', 'h2')]
        if q == 18:
            sops += [('ln', 'h0'), ('rexp', 'h0')]
    sops += [('hists', n) for n in range(len(S_PASSES))]
    sops.append(('lnd',))
    # pool ops
    pops = [('memset',)]

    tidx = {op: i + 1 for i, op in enumerate(tops)}
    vidx = {op: i + 1 for i, op in enumerate(vops)}
    sidx = {op: i + 1 for i, op in enumerate(sops)}
    pidx = {op: i + 1 for i, op in enumerate(pops)}

    from contextlib import ExitStack
    with ExitStack() as es:
        def sb(name, shape, dtype=F32):
            return es.enter_context(nc.sbuf_tensor(name, shape, dtype))

        preds_sb = sb("preds_sb", [49, NCH * 96], BF16)
        uyt_sb = sb("uyt_sb", [49, 192], BF16)
        ux_sb = sb("ux_sb", [96, 384], BF16)
        uxT_sb = sb("uxT_sb", [128, 3 * 96], BF16)
        tf_sb = sb("tf_sb", [128, 3 * 576], BF16)
        t1_sb = sb("t1_sb", [96, NCH * 192], BF16)
        e_sb = sb("e_sb", [128, NCH * 576], BF16)
        s_sb = sb("s_sb", [128, 4 * 576], BF16)
        r_sb = sb("r_sb", [128, 3 * 576], BF16)
        ln_sb = sb("ln_sb", [128, 576])
        fg_sb = sb("fg_sb", [128, 12 * 576], BF16)
        xb_sb = sb("xb_sb", [128, 12 * 576], BF16)
        cst_sb = sb("cst_sb", [128, 4])
        junkv_sb = sb("junkv_sb", [128, 576], BF16)
        junks_sb = sb("junks_sb", [128, 576], BF16)
        acc_sb = sb("acc_sb", [128, ACC_W])

        ps1 = [es.enter_context(nc.psum_tensor(f"ps1{i}", [96, 384], F32)) for i in range(3)]
        ps2 = [es.enter_context(nc.psum_tensor(f"ps2{i}", [128, 384], F32)) for i in range(3)]
        psA = [es.enter_context(nc.psum_tensor(f"psA{i}", [96, 192], F32)) for i in range(2)]

        for i, val in enumerate(BIAS_VALS):
            nc.const_aps.aps[(F32, val)] = cst_sb[:, i: i + 1]

        dmaP = es.enter_context(nc.semaphore("dmaP"))
        dmaU = es.enter_context(nc.semaphore("dmaU"))
        dmaX = es.enter_context(nc.semaphore("dmaX"))
        dmaXT = es.enter_context(nc.semaphore("dmaXT"))
        dmaT = es.enter_context(nc.semaphore("dmaT"))
        dmaC = es.enter_context(nc.semaphore("dmaC"))
        t_sem = es.enter_context(nc.semaphore("t_sem"))
        v_sem = es.enter_context(nc.semaphore("v_sem"))
        s_sem = es.enter_context(nc.semaphore("s_sem"))
        p_sem = es.enter_context(nc.semaphore("p_sem"))
        odma = es.enter_context(nc.semaphore("odma"))

        SEMS = {'t': t_sem, 'v': v_sem, 's': s_sem, 'p': p_sem,
                'P': dmaP, 'U': dmaU, 'X': dmaX, 'XT': dmaXT, 'T': dmaT,
                'C': dmaC}
        IDX = {'t': tidx, 'v': vidx, 's': sidx, 'p': pidx}

        def mk_waiter(eng):
            seen = {}
            def wait(dom, tag=None):
                sem = SEMS[dom]
                n = 16 if tag is None else IDX[dom][tag]
                if seen.get(dom, 0) >= n:
                    return
                seen[dom] = n
                eng.wait_ge(sem, n)
            return wait

        # slice helpers
        def e_ch(c):
            return e_sb[:, 576 * c: 576 * (c + 1)]

        def t1_ch(c):
            return t1_sb[0:96, 192 * c: 192 * (c + 1)]

        def fg_t(ci):
            return fg_sb[:, 576 * ci: 576 * (ci + 1)]

        def xb_t(ci):
            return xb_sb[:, 576 * ci: 576 * (ci + 1)]

        def s_t(h):
            return s_sb[:, S_OFF[h]: S_OFF[h] + 576]

        def r_t(h):
            return r_sb[:, R_OFF[h]: R_OFF[h] + 576]

        def tf_head(ci):
            head = CL[ci][0]
            off = {"h0": 0, "h1": 576, "h2": 1152}[head]
            return tf_sb[:, off: off + 576]

        def acc_col(col, rows=128):
            return acc_sb[0:rows, col: col + 1]

        # exp bank boundary helpers
        def expbank_of_chunk(m):
            return m // 2

        def e_ready_bank(c):
            """exp bank index that completes channel c's tile."""
            return expbank_of_chunk(3 * c + 2)

        with nc.Block() as block:

            @block.sync
            def _(sync):
                sync.dma_start(out=preds_sb[:, :], in_=p_preds[:, :]).then_inc(dmaP, 16)
                sync.dma_start(out=uyt_sb[:, :], in_=p_uyt[:, :]).then_inc(dmaU, 16)
                sync.dma_start(out=tf_sb[:, :], in_=p_tgt[:, :]).then_inc(dmaT, 16)
                sync.dma_start(out=ux_sb[:, :], in_=p_ux[:, :]).then_inc(dmaX, 16)
                sync.dma_start(out=uxT_sb[:, :], in_=p_uxT[:, :]).then_inc(dmaXT, 16)
                sync.dma_start(out=cst_sb[:, :], in_=p_cst[:, :]).then_inc(dmaC, 16)
                sync.wait_ge(v_sem, len(vops))
                sync.wait_ge(s_sem, len(sops))
                sync.wait_ge(p_sem, len(pops))
                sync.dma_start(out=p_acc[:, :], in_=acc_sb[:, :]).then_inc(odma, 16)
                sync.wait_ge(odma, 16)

            @block.tensor
            def _(tensor):
                wait = mk_waiter(tensor)
                for op in tops:
                    if op[0] == 'mm1':
                        c = op[1]
                        if c == 0:
                            wait('P'); wait('U')
                        j = c // 2
                        if c % 2 == 0 and j >= 3:
                            wait('v', ('copy1', j - 3))
                        tensor.matmul(
                            ps1[j % 3][0:96, 192 * (c % 2): 192 * (c % 2) + 192],
                            preds_sb[0:49, 96 * c: 96 * (c + 1)],
                            uyt_sb[0:49, 0:192],
                            start=True, stop=True,
                        ).then_inc(t_sem)
                    elif op[0] == 'mm2':
                        m = op[1]
                        c, k = divmod(m, 3)
                        q = m // 2
                        if m == 0:
                            wait('X')
                        wait('v', ('copy1', c // 2))
                        if q >= 3 and m % 2 == 0:
                            wait('s', ('exp', q - 3))
                        tensor.matmul(
                            ps2[q % 3][0:128, 192 * (m % 2): 192 * (m % 2) + 192],
                            ux_sb[0:96, 128 * k: 128 * (k + 1)],
                            t1_ch(c),
                            start=True, stop=True,
                        ).then_inc(t_sem)
                    else:  # A matmul
                        _, ci, k = op
                        if k == 0:
                            wait('XT')
                            wait('v', ('fg', ci))
                            if ci >= 7:
                                wait('v', ('zfd', ci - 2))
                        tensor.matmul(
                            psA[ci % 2][0:96, 0:192],
                            uxT_sb[0:128, 96 * k: 96 * (k + 1)],
                            fg_sb[:, 576 * ci + 192 * k: 576 * ci + 192 * (k + 1)],
                            start=(k == 0), stop=(k == 2),
                        ).then_inc(t_sem)

            @block.scalar
            def _(scalar):
                wait = mk_waiter(scalar)
                for op in sops:
                    if op[0] == 'warm':
                        # touch the Exp/Ln act table so the 1.3us table load
                        # happens during DMA startup, off the critical path
                        scalar.activation(junks_sb[0:1, 0:1], junks_sb[0:1, 0:1],
                                          AF.Exp).then_inc(s_sem)
                    elif op[0] == 'exp':
                        q = op[1]
                        w = 384 if q < 28 else 192
                        wait('t', ('mm2', min(2 * q + 1, 56)))
                        scalar.activation(
                            e_sb[:, 384 * q: 384 * q + w],
                            ps2[q % 3][0:128, 0:w], AF.Exp,
                        ).then_inc(s_sem)
                    elif op[0] == 'ln':
                        h = op[1]
                        if h == 'h1':
                            wait('v', ('Sadd', 'h1', 1))
                            scalar.activation(ln_sb[:, :], s_t('h1'), AF.Ln).then_inc(s_sem)
                        elif h == 'h2':
                            wait('v', ('Sadd', 'h2', 0))
                            scalar.activation(ln_sb[:, :], s_t('h2'), AF.Ln).then_inc(s_sem)
                        else:
                            wait('v', ('Sadd', 'h0', 5))
                            wait('p', ('memset',))
                            scalar.activation(
                                ln_sb[:, :], s_t('h0'), AF.Ln,
                                accum_out=acc_col(COL_LNS0),
                            ).then_inc(s_sem)
                    elif op[0] == 'rexp':
                        h = op[1]
                        scalar.activation(r_t(h), ln_sb[:, :], AF.Exp, scale=-1.0).then_inc(s_sem)
                    elif op[0] == 'lnd':
                        wait('v', ('dsnSv', 5))
                        scalar.activation(
                            junks_sb[:, :], s_t('d'), AF.Ln,
                            accum_out=acc_col(COL_LNSD),
                        ).then_inc(s_sem)
                    else:  # hists
                        n = op[1]
                        ci, side, j = S_PASSES[n]
                        t = thr_of(ci)[j]
                        wait('C')
                        wait('v', ('x', ci))
                        scalar.activation(
                            junks_sb[:, :], xb_t(ci), AF.Relu,
                            bias=-t, scale=(1.0 if side == 'F' else -1.0),
                            accum_out=acc_col(col_of(ci, side, j)),
                        ).then_inc(s_sem)

            @block.vector
            def _(vector):
                wait = mk_waiter(vector)
                first_fg = True
                first_acc = True
                for op in vops:
                    if op[0] == 'copy1':
                        j = op[1]
                        w = 384 if j < 9 else 192
                        wait('t', ('mm1', min(2 * j + 1, 18)))
                        vector.tensor_copy(
                            t1_sb[0:96, 384 * j: 384 * j + w],
                            ps1[j % 3][0:96, 0:w],
                        ).then_inc(v_sem)
                    elif op[0] == 'fg':
                        ci = op[1]
                        if first_fg:
                            wait('T')
                            first_fg = False
                        head, c = CL[ci]
                        vector.tensor_scalar(
                            fg_t(ci), tf_head(ci), float(c), 0.0,
                            OP.is_equal, OP.add,
                        ).then_inc(v_sem)
                    elif op[0] == 'Sadd':
                        _, h, i = op
                        if h == 'h1':
                            if i == 0:
                                wait('s', ('exp', e_ready_bank(1)))
                                vector.tensor_add(s_t('h1'), e_ch(0), e_ch(1)).then_inc(v_sem)
                            else:
                                wait('s', ('exp', e_ready_bank(2)))
                                vector.tensor_add(s_t('h1'), s_t('h1'), e_ch(2)).then_inc(v_sem)
                        elif h == 'h2':
                            wait('s', ('exp', e_ready_bank(4)))
                            vector.tensor_add(s_t('h2'), e_ch(3), e_ch(4)).then_inc(v_sem)
                        else:
                            if i == 0:
                                wait('s', ('exp', e_ready_bank(6)))
                                vector.tensor_add(s_t('h0'), e_ch(5), e_ch(6)).then_inc(v_sem)
                            else:
                                wait('s', ('exp', e_ready_bank(6 + i)))
                                vector.tensor_add(s_t('h0'), s_t('h0'), e_ch(6 + i)).then_inc(v_sem)
                    elif op[0] == 'p':
                        ci = op[1]
                        head = CL[ci][0]
                        wait('s', ('rexp', head))
                        ch = chan_of(ci)
                        vector.tensor_mul(e_ch(ch), e_ch(ch), r_t(head)).then_inc(v_sem)
                    elif op[0] == 'x':
                        ci = op[1]
                        vector.tensor_tensor(
                            xb_t(ci), fg_t(ci), e_ch(chan_of(ci)), OP.subtract,
                        ).then_inc(v_sem)
                    elif op[0] == 'histv':
                        n = op[1]
                        ci, side, j = V_PASSES[n]
                        t = thr_of(ci)[j]
                        if first_acc:
                            wait('p', ('memset',))
                            first_acc = False
                        cl = acc_col(col_of(ci, side, j))
                        if side == 'F':
                            vector.tensor_scalar(junkv_sb[:, :], xb_t(ci), t, 0.0,
                                                 OP.max, OP.add, accum_out=cl).then_inc(v_sem)
                        else:
                            vector.tensor_scalar(junkv_sb[:, :], xb_t(ci), -t, 0.0,
                                                 OP.min, OP.add, accum_out=cl).then_inc(v_sem)
                    elif op[0] == 'zfh':
                        ci = op[1]
                        wait('t', ('A', ci, 2))
                        if first_acc:
                            wait('p', ('memset',))
                            first_acc = False
                        vector.scalar_tensor_tensor(
                            junkv_sb[0:96, 0:192], t1_ch(ci), 1.0,
                            psA[ci % 2][0:96, 0:192], OP.mult, OP.mult,
                            accum_out=acc_col(col_zf(ci, 0), rows=96),
                        ).then_inc(v_sem)
                    elif op[0] == 'zfd':
                        ci = op[1]
                        vector.scalar_tensor_tensor(
                            junkv_sb[0:96, 0:192], t1_ch(ci + 7), 1.0,
                            psA[ci % 2][0:96, 0:192], OP.mult, OP.mult,
                            accum_out=acc_col(col_zf(ci, 1), rows=96),
                        ).then_inc(v_sem)
                    else:  # dsnSv
                        i = op[1]
                        if i == 0:
                            wait('s', ('exp', e_ready_bank(13)))
                            vector.tensor_add(s_t('d'), e_ch(12), e_ch(13)).then_inc(v_sem)
                        else:
                            wait('s', ('exp', e_ready_bank(13 + i)))
                            vector.tensor_add(s_t('d'), s_t('d'), e_ch(13 + i)).then_inc(v_sem)

            @block.gpsimd
            def _(gpsimd):
                wait = mk_waiter(gpsimd)
                for op in pops:
                    if op[0] == 'memset':
                        gpsimd.memset(acc_sb[:, :], 0.0).then_inc(p_sem)


    return nc


# ---------------------------------------------------------------- host side --

def _interp_weights():
    s = np.linspace(np.float32(0.0), np.float32(95.0), 384).astype(np.float32)
    i0 = np.clip(np.floor(s).astype(np.int64), 0, 94)
    t = (s - i0).astype(np.float32)
    return i0, t


_CHAN_SRC = ([("preds1", c) for c in range(3)] + [("preds2", c) for c in range(2)]
             + [("preds0", c) for c in range(7)] + [("preds_dsn", c) for c in range(7)])


def _prep_core(inputs, core):
    b, half = core // 2, core % 2
    r0 = half * 192
    cy0 = 0 if half == 0 else 47
    i0, t = _interp_weights()

    uyt = np.zeros((49, 192), np.float32)
    for fy in range(192):
        f = r0 + fy
        uyt[i0[f] - cy0, fy] += np.float32(1.0) - t[f]
        uyt[i0[f] + 1 - cy0, fy] += t[f]

    ux = np.zeros((96, 384), np.float32)
    for X in range(384):
        ux[i0[X], X] += np.float32(1.0) - t[X]
        ux[i0[X] + 1, X] += t[X]
    ux = ux.astype(BF)
    uxT = np.zeros((128, 3 * 96), BF)
    for k in range(3):
        uxT[:, 96 * k: 96 * (k + 1)] = ux[:, 128 * k: 128 * (k + 1)].T

    pa = np.zeros((49, NCH * 96), BF)
    for idx, (key, ch) in enumerate(_CHAN_SRC):
        pa[:, idx * 96: (idx + 1) * 96] = inputs[key][b, ch, cy0: cy0 + 49, :].astype(BF)

    tg = np.zeros((128, 3 * 576), BF)
    for h, key in enumerate(["targets0", "targets1", "targets2"]):
        th = inputs[key][b, r0: r0 + 192, :]
        tg[:, 576 * h: 576 * (h + 1)] = (
            th.reshape(192, 3, 128).transpose(2, 1, 0).reshape(128, 576)
        ).astype(BF)

    cst = np.tile(np.asarray(BIAS_VALS + [0.0], np.float32), (128, 1))
    return {"preds": pa, "uyt": uyt.astype(BF), "ux": ux, "uxT": uxT, "tgt": tg,
            "cst": cst}


def _ncs_core(inputs, core):
    """Exact per-class pixel counts for this shard, from integer targets."""
    b, half = core // 2, core % 2
    r0 = half * 192
    ncs = []
    for ci, (head, c) in enumerate(CL):
        key = {"h1": "targets1", "h2": "targets2", "h0": "targets0"}[head]
        lab = inputs[key][b, r0: r0 + 192, :]
        ncs.append(float((lab == c).sum()))
    return ncs


def _finale(accs, ncs_all):
    lov_total = 0.0
    ce0_num = 0.0
    ced_num = 0.0
    for acc, ncs in zip(accs, ncs_all):
        cs = acc.astype(np.float64).sum(axis=0)
        head_lov = {"h1": [], "h2": [], "h0": []}
        for ci, (head, c) in enumerate(CL):
            thr = thr_of(ci)
            K = len(thr)
            base = 16 * ci
            n_c = ncs[ci]
            TF, TB = [], []
            for j, t in enumerate(thr):
                cF = cs[base + 6 + j]
                cB = cs[base + j]
                if (ci, 'F', j) in SCALAR_SET:
                    TF.append(cF)
                else:
                    TF.append(cF - N_PIX * t)
                if (ci, 'B', j) in SCALAR_SET:
                    TB.append(cB)
                else:
                    TB.append(-cB - N_PIX * t)
            TF.append(0.0)
            TB.append(0.0)
            if n_c < 0.5:
                continue
            ts_ext = list(thr) + [1.0]
            L = 0.0
            for j in range(K):
                IF = TF[j] - TF[j + 1]
                IB = TB[j] - TB[j + 1]
                d = ts_ext[j + 1] - ts_ext[j]
                L += (IF + IB) / (n_c + IB / d)
            head_lov[head].append(L)
        for head, w in (("h0", 1.0), ("h1", 0.4), ("h2", 0.4)):
            vals = head_lov[head]
            lov_total += w * (sum(vals) / max(len(vals), 1))
        ce0_num += cs[COL_LNS0] - sum(cs[16 * ci + 13] for ci in range(5, 12))
        ced_num += cs[COL_LNSD] - sum(cs[16 * ci + 14] for ci in range(5, 12))
    return ce0_num / P_GLOBAL + 0.4 * (ced_num / P_GLOBAL) + lov_total / 8.0


_NC_CACHE = None


def kernel(**inputs):
    global _NC_CACHE
    inputs = {k: np.asarray(v) for k, v in inputs.items()}
    if _NC_CACHE is None:
        _NC_CACHE = build_kernel()
    nc = _NC_CACHE
    in_maps = [_prep_core(inputs, core) for core in range(8)]
    res = run_bass_kernel_spmd(nc, in_maps, core_ids=list(range(8)))
    accs = [np.asarray(res.results[c]["acc"], dtype=np.float32) for c in range(8)]
    ncs_all = [_ncs_core(inputs, c) for c in range(8)]
    loss = _finale(accs, ncs_all)
    return np.asarray(loss, dtype=np.float32)


# revision 16
# speedup vs baseline: 3.0381x; 1.0862x over previous
"""Trainium2 distributed kernel for ABRLovaszCELoss (8 NeuronCores).

Strategy (v4)
-------------
Data-parallel over (batch, row-half): core i handles batch b=i//2, fine rows
[192*(i%2), 192*(i%2)+192) of the 384x384 target grid (73728 pixels/core).

Per core, fully on-device (all-bf16 datapath):
- bilinear align_corners upsample 96->384 of all 19 logit channels
  (order head1:3, head2:2, head0:7, dsn:7) as two bf16 PE matmuls per
  channel; pixel layout [128 part = X%128, 576 free = 192*(X//128) + fy].
- stage-1 PSUM pairs copied to SBUF bf16 by vector; stage-2 pairs exp'd
  384-wide on scalar into one contiguous e_all tile.
- softmax: per-head S sums on vector (dsn S on pool), r = exp(-ln S) on
  scalar; the head0 Ln pass also accumulates CE's sum(ln S); p = e*r
  in-place; x = fg - p; fg masks via fast tensor_scalar is_equal.
- CE's sum(z*[tgt==c]) via linearity of the interpolation:
  <t1_c, A_c> where A_c = ux^T-adjoint of the fg mask (3 accumulating PE
  matmuls per head0 class), consumed by tiny [96,192] stt accumulations.
  n_c is computed exactly on the host from the integer targets.
- Lovasz-Softmax per shard via exact relu tail-integrals (no sort):
  TF_j = sum relu(x - t_j), TB_j = sum relu(-x - t_j) at bf16-exact
  thresholds; per-segment integrals by differencing on the host, and
  L_c = sum_j (IF_j + IB_j) / (n_c + IB_j/dt_j).
  Histogram passes split across vector (tensor_scalar add-reduce accum),
  scalar (Relu+accum), and pool (vector-prepped relu tiles + XYZWC
  tensor_reduce).
- the [128,256] accumulator tile is DMA'd out per core; the final
  per-class differencing/reciprocal algebra and the 8-shard reduction
  happen on the host during gather/unshard (exact fp64).
"""

import numpy as np
import ml_dtypes

import concourse.bass as bass
import concourse.mybir as mybir
from concourse.bass_utils import run_bass_kernel_spmd

F32 = mybir.dt.float32
BF16 = mybir.dt.bfloat16
AF = mybir.ActivationFunctionType
OP = mybir.AluOpType
AX = mybir.AxisListType
BF = ml_dtypes.bfloat16

NCH = 19
N_PIX = 73728
P_GLOBAL = 4 * 384 * 384

# channel order: head1 (3), head2 (2), head0 (7), dsn (7)
THR12 = (0.0, 0.34375, 0.671875)   # bf16-exact ~ j/3
THR0 = (0.0, 0.5)

# lovasz classes in "CL" order
CL = ([("h1", c) for c in range(3)] + [("h2", c) for c in range(2)]
      + [("h0", c) for c in range(7)])
HEAD_CH0 = {"h1": 0, "h2": 3, "h0": 5, "d": 12}
S_OFF = {"h1": 0, "h2": 576, "h0": 1152, "d": 1728}
R_OFF = {"h1": 0, "h2": 576, "h0": 1152}


def chan_of(ci):
    head, c = CL[ci]
    return HEAD_CH0[head] + c


def thr_of(ci):
    return THR12 if ci < 5 else THR0


# histogram pass assignment (side 'B' = TB via min/relu(-x-t), 'F' = TF)
# vector: h1+h2 (tensor_scalar add-reduce); scalar: all h0 classes
V_PASSES = [(ci, s, j) for ci in range(5) for s in 'BF' for j in range(3)]
S_PASSES = [(ci, s, j) for ci in range(5, 12) for s in 'BF' for j in range(2)]
SCALAR_SET = set(S_PASSES)

ACC_W = 256
COL_LNS0 = 192
COL_LNSD = 193


def col_of(ci, side, j):
    return 16 * ci + (j if side == 'B' else 6 + j)


def col_zf(ci, which):   # which: 0 = head0, 1 = dsn
    return 16 * ci + 13 + which


N_JR = 6   # jr ring slots


BIAS_VALS = sorted({-t for t in THR0[1:]} | {-t for t in THR12[1:]})


def build_kernel():
    nc = bass.Bass()

    p_cst = nc.declare_dram_parameter("cst", [128, 4], F32, isOutput=False)
    p_preds = nc.declare_dram_parameter("preds", [49, NCH * 96], BF16, isOutput=False)
    p_uyt = nc.declare_dram_parameter("uyt", [49, 192], BF16, isOutput=False)
    p_ux = nc.declare_dram_parameter("ux", [96, 384], BF16, isOutput=False)
    p_uxT = nc.declare_dram_parameter("uxT", [128, 3 * 96], BF16, isOutput=False)
    p_tgt = nc.declare_dram_parameter("tgt", [128, 3 * 576], BF16, isOutput=False)
    p_acc = nc.declare_dram_parameter("acc", [128, ACC_W], F32, isOutput=True)

    # ---------------- static program-order op lists (for cross-engine idx) --
    # tensor ops
    # mm2 chunks 0..5 (ps2 pairs 0..2, no exp wait) are emitted early so the
    # scalar exp stream starts while the remaining mm1s are still flowing;
    # all exp-gated mm2 chunks come after every mm1 (V's stage-1 copies must
    # never transitively depend on the scalar stream).  All A matmuls come
    # after the last stage-2 chunk: their waits on V's zfg consumption must
    # never stall the exp stream.
    tops = []
    for c in range(6):
        tops.append(('mm1', c))
    for m in range(6):
        tops.append(('mm2', m))
    for c in range(6, NCH):
        tops.append(('mm1', c))
    for m in range(6, 57):
        tops.append(('mm2', m))
    for ci in range(5, 12):
        for k in range(3):
            tops.append(('A', ci, k))
    # vector ops: copies first (paces the mm1/mm2 streams); masks and hist
    # passes act as filler around the exp-gated S adds; h0's S adds go as
    # early as possible since scalar's ln/rexp (and so p0/x0 and the scalar
    # hist tail) gate on them; the zfg stts go last (they only feed the acc
    # output and would otherwise stall V on the late A matmuls).
    vops = [('copy1', j) for j in range(10)]
    vops += [('Sadd', 'h1', 0), ('Sadd', 'h1', 1)]
    vops += [('fg', i) for i in range(12)]
    vops += [('Sadd', 'h2', 0)]
    vops += [('p', ci) for ci in range(3)]
    vops += [('x', ci) for ci in range(3)]
    vops += [('p', 3), ('p', 4), ('x', 3), ('x', 4)]
    vops += [('histv', 0), ('histv', 1), ('histv', 2), ('histv', 3)]
    vops += [('Sadd', 'h0', i) for i in range(6)]
    vops += [('histv', 4), ('histv', 5)]   # filler while rexp(h0) lands
    vops += [('p', ci) for ci in range(5, 12)]
    vops += [('x', ci) for ci in range(5, 12)]
    vops += [('dsnSv', i) for i in range(6)]
    vops += [('histv', i) for i in range(6, len(V_PASSES))]
    for ci in range(5, 12):
        vops += [('zfh', ci), ('zfd', ci)]
    # scalar ops
    sops = [('warm',)]
    for q in range(29):
        sops.append(('exp', q))
        if q == 5:
            sops += [('ln', 'h1'), ('rexp', 'h1')]
        if q == 8:
            sops += [('ln', 'h2'), ('rexp', 'h2')]
        if q == 18:
            sops += [('ln', 'h0'), ('rexp', 'h0')]
    sops += [('hists', n) for n in range(len(S_PASSES))]
    sops.append(('lnd',))
    # pool ops
    pops = [('memset',)]

    tidx = {op: i + 1 for i, op in enumerate(tops)}
    vidx = {op: i + 1 for i, op in enumerate(vops)}
    sidx = {op: i + 1 for i, op in enumerate(sops)}
    pidx = {op: i + 1 for i, op in enumerate(pops)}

    from contextlib import ExitStack
    with ExitStack() as es:
        def sb(name, shape, dtype=F32):
            return es.enter_context(nc.sbuf_tensor(name, shape, dtype))

        preds_sb = sb("preds_sb", [49, NCH * 96], BF16)
        uyt_sb = sb("uyt_sb", [49, 192], BF16)
        ux_sb = sb("ux_sb", [96, 384], BF16)
        uxT_sb = sb("uxT_sb", [128, 3 * 96], BF16)
        tf_sb = sb("tf_sb", [128, 3 * 576], BF16)
        t1_sb = sb("t1_sb", [96, NCH * 192], BF16)
        e_sb = sb("e_sb", [128, NCH * 576], BF16)
        s_sb = sb("s_sb", [128, 4 * 576], BF16)
        r_sb = sb("r_sb", [128, 3 * 576], BF16)
        ln_sb = sb("ln_sb", [128, 576])
        fg_sb = sb("fg_sb", [128, 12 * 576], BF16)
        xb_sb = sb("xb_sb", [128, 12 * 576], BF16)
        cst_sb = sb("cst_sb", [128, 4])
        junkv_sb = sb("junkv_sb", [128, 576], BF16)
        junks_sb = sb("junks_sb", [128, 576], BF16)
        acc_sb = sb("acc_sb", [128, ACC_W])

        ps1 = [es.enter_context(nc.psum_tensor(f"ps1{i}", [96, 384], F32)) for i in range(3)]
        ps2 = [es.enter_context(nc.psum_tensor(f"ps2{i}", [128, 384], F32)) for i in range(3)]
        psA = [es.enter_context(nc.psum_tensor(f"psA{i}", [96, 192], F32)) for i in range(2)]

        for i, val in enumerate(BIAS_VALS):
            nc.const_aps.aps[(F32, val)] = cst_sb[:, i: i + 1]

        dmaP = es.enter_context(nc.semaphore("dmaP"))
        dmaU = es.enter_context(nc.semaphore("dmaU"))
        dmaX = es.enter_context(nc.semaphore("dmaX"))
        dmaXT = es.enter_context(nc.semaphore("dmaXT"))
        dmaT = es.enter_context(nc.semaphore("dmaT"))
        dmaC = es.enter_context(nc.semaphore("dmaC"))
        t_sem = es.enter_context(nc.semaphore("t_sem"))
        v_sem = es.enter_context(nc.semaphore("v_sem"))
        s_sem = es.enter_context(nc.semaphore("s_sem"))
        p_sem = es.enter_context(nc.semaphore("p_sem"))
        odma = es.enter_context(nc.semaphore("odma"))

        SEMS = {'t': t_sem, 'v': v_sem, 's': s_sem, 'p': p_sem,
                'P': dmaP, 'U': dmaU, 'X': dmaX, 'XT': dmaXT, 'T': dmaT,
                'C': dmaC}
        IDX = {'t': tidx, 'v': vidx, 's': sidx, 'p': pidx}

        def mk_waiter(eng):
            seen = {}
            def wait(dom, tag=None):
                sem = SEMS[dom]
                n = 16 if tag is None else IDX[dom][tag]
                if seen.get(dom, 0) >= n:
                    return
                seen[dom] = n
                eng.wait_ge(sem, n)
            return wait

        # slice helpers
        def e_ch(c):
            return e_sb[:, 576 * c: 576 * (c + 1)]

        def t1_ch(c):
            return t1_sb[0:96, 192 * c: 192 * (c + 1)]

        def fg_t(ci):
            return fg_sb[:, 576 * ci: 576 * (ci + 1)]

        def xb_t(ci):
            return xb_sb[:, 576 * ci: 576 * (ci + 1)]

        def s_t(h):
            return s_sb[:, S_OFF[h]: S_OFF[h] + 576]

        def r_t(h):
            return r_sb[:, R_OFF[h]: R_OFF[h] + 576]

        def tf_head(ci):
            head = CL[ci][0]
            off = {"h0": 0, "h1": 576, "h2": 1152}[head]
            return tf_sb[:, off: off + 576]

        def acc_col(col, rows=128):
            return acc_sb[0:rows, col: col + 1]

        # exp bank boundary helpers
        def expbank_of_chunk(m):
            return m // 2

        def e_ready_bank(c):
            """exp bank index that completes channel c's tile."""
            return expbank_of_chunk(3 * c + 2)

        with nc.Block() as block:

            @block.sync
            def _(sync):
                sync.dma_start(out=preds_sb[:, :], in_=p_preds[:, :]).then_inc(dmaP, 16)
                sync.dma_start(out=uyt_sb[:, :], in_=p_uyt[:, :]).then_inc(dmaU, 16)
                sync.dma_start(out=tf_sb[:, :], in_=p_tgt[:, :]).then_inc(dmaT, 16)
                sync.dma_start(out=ux_sb[:, :], in_=p_ux[:, :]).then_inc(dmaX, 16)
                sync.dma_start(out=uxT_sb[:, :], in_=p_uxT[:, :]).then_inc(dmaXT, 16)
                sync.dma_start(out=cst_sb[:, :], in_=p_cst[:, :]).then_inc(dmaC, 16)
                sync.wait_ge(v_sem, len(vops))
                sync.wait_ge(s_sem, len(sops))
                sync.wait_ge(p_sem, len(pops))
                sync.dma_start(out=p_acc[:, :], in_=acc_sb[:, :]).then_inc(odma, 16)
                sync.wait_ge(odma, 16)

            @block.tensor
            def _(tensor):
                wait = mk_waiter(tensor)
                for op in tops:
                    if op[0] == 'mm1':
                        c = op[1]
                        if c == 0:
                            wait('P'); wait('U')
                        j = c // 2
                        if c % 2 == 0 and j >= 3:
                            wait('v', ('copy1', j - 3))
                        tensor.matmul(
                            ps1[j % 3][0:96, 192 * (c % 2): 192 * (c % 2) + 192],
                            preds_sb[0:49, 96 * c: 96 * (c + 1)],
                            uyt_sb[0:49, 0:192],
                            start=True, stop=True,
                        ).then_inc(t_sem)
                    elif op[0] == 'mm2':
                        m = op[1]
                        c, k = divmod(m, 3)
                        q = m // 2
                        if m == 0:
                            wait('X')
                        wait('v', ('copy1', c // 2))
                        if q >= 3 and m % 2 == 0:
                            wait('s', ('exp', q - 3))
                        tensor.matmul(
                            ps2[q % 3][0:128, 192 * (m % 2): 192 * (m % 2) + 192],
                            ux_sb[0:96, 128 * k: 128 * (k + 1)],
                            t1_ch(c),
                            start=True, stop=True,
                        ).then_inc(t_sem)
                    else:  # A matmul
                        _, ci, k = op
                        if k == 0:
                            wait('XT')
                            wait('v', ('fg', ci))
                            if ci >= 7:
                                wait('v', ('zfd', ci - 2))
                        tensor.matmul(
                            psA[ci % 2][0:96, 0:192],
                            uxT_sb[0:128, 96 * k: 96 * (k + 1)],
                            fg_sb[:, 576 * ci + 192 * k: 576 * ci + 192 * (k + 1)],
                            start=(k == 0), stop=(k == 2),
                        ).then_inc(t_sem)

            @block.scalar
            def _(scalar):
                wait = mk_waiter(scalar)
                for op in sops:
                    if op[0] == 'warm':
                        # touch the Exp/Ln act table so the 1.3us table load
                        # happens during DMA startup, off the critical path
                        scalar.activation(junks_sb[0:1, 0:1], junks_sb[0:1, 0:1],
                                          AF.Exp).then_inc(s_sem)
                    elif op[0] == 'exp':
                        q = op[1]
                        w = 384 if q < 28 else 192
                        wait('t', ('mm2', min(2 * q + 1, 56)))
                        scalar.activation(
                            e_sb[:, 384 * q: 384 * q + w],
                            ps2[q % 3][0:128, 0:w], AF.Exp,
                        ).then_inc(s_sem)
                    elif op[0] == 'ln':
                        h = op[1]
                        if h == 'h1':
                            wait('v', ('Sadd', 'h1', 1))
                            scalar.activation(ln_sb[:, :], s_t('h1'), AF.Ln).then_inc(s_sem)
                        elif h == 'h2':
                            wait('v', ('Sadd', 'h2', 0))
                            scalar.activation(ln_sb[:, :], s_t('h2'), AF.Ln).then_inc(s_sem)
                        else:
                            wait('v', ('Sadd', 'h0', 5))
                            wait('p', ('memset',))
                            scalar.activation(
                                ln_sb[:, :], s_t('h0'), AF.Ln,
                                accum_out=acc_col(COL_LNS0),
                            ).then_inc(s_sem)
                    elif op[0] == 'rexp':
                        h = op[1]
                        scalar.activation(r_t(h), ln_sb[:, :], AF.Exp, scale=-1.0).then_inc(s_sem)
                    elif op[0] == 'lnd':
                        wait('v', ('dsnSv', 5))
                        scalar.activation(
                            junks_sb[:, :], s_t('d'), AF.Ln,
                            accum_out=acc_col(COL_LNSD),
                        ).then_inc(s_sem)
                    else:  # hists
                        n = op[1]
                        ci, side, j = S_PASSES[n]
                        t = thr_of(ci)[j]
                        wait('C')
                        wait('v', ('x', ci))
                        scalar.activation(
                            junks_sb[:, :], xb_t(ci), AF.Relu,
                            bias=-t, scale=(1.0 if side == 'F' else -1.0),
                            accum_out=acc_col(col_of(ci, side, j)),
                        ).then_inc(s_sem)

            @block.vector
            def _(vector):
                wait = mk_waiter(vector)
                first_fg = True
                first_acc = True
                for op in vops:
                    if op[0] == 'copy1':
                        j = op[1]
                        w = 384 if j < 9 else 192
                        wait('t', ('mm1', min(2 * j + 1, 18)))
                        vector.tensor_copy(
                            t1_sb[0:96, 384 * j: 384 * j + w],
                            ps1[j % 3][0:96, 0:w],
                        ).then_inc(v_sem)
                    elif op[0] == 'fg':
                        ci = op[1]
                        if first_fg:
                            wait('T')
                            first_fg = False
                        head, c = CL[ci]
                        vector.tensor_scalar(
                            fg_t(ci), tf_head(ci), float(c), 0.0,
                            OP.is_equal, OP.add,
                        ).then_inc(v_sem)
                    elif op[0] == 'Sadd':
                        _, h, i = op
                        if h == 'h1':
                            if i == 0:
                                wait('s', ('exp', e_ready_bank(1)))
                                vector.tensor_add(s_t('h1'), e_ch(0), e_ch(1)).then_inc(v_sem)
                            else:
                                wait('s', ('exp', e_ready_bank(2)))
                                vector.tensor_add(s_t('h1'), s_t('h1'), e_ch(2)).then_inc(v_sem)
                        elif h == 'h2':
                            wait('s', ('exp', e_ready_bank(4)))
                            vector.tensor_add(s_t('h2'), e_ch(3), e_ch(4)).then_inc(v_sem)
                        else:
                            if i == 0:
                                wait('s', ('exp', e_ready_bank(6)))
                                vector.tensor_add(s_t('h0'), e_ch(5), e_ch(6)).then_inc(v_sem)
                            else:
                                wait('s', ('exp', e_ready_bank(6 + i)))
                                vector.tensor_add(s_t('h0'), s_t('h0'), e_ch(6 + i)).then_inc(v_sem)
                    elif op[0] == 'p':
                        ci = op[1]
                        head = CL[ci][0]
                        wait('s', ('rexp', head))
                        ch = chan_of(ci)
                        vector.tensor_mul(e_ch(ch), e_ch(ch), r_t(head)).then_inc(v_sem)
                    elif op[0] == 'x':
                        ci = op[1]
                        vector.tensor_tensor(
                            xb_t(ci), fg_t(ci), e_ch(chan_of(ci)), OP.subtract,
                        ).then_inc(v_sem)
                    elif op[0] == 'histv':
                        n = op[1]
                        ci, side, j = V_PASSES[n]
                        t = thr_of(ci)[j]
                        if first_acc:
                            wait('p', ('memset',))
                            first_acc = False
                        cl = acc_col(col_of(ci, side, j))
                        if side == 'F':
                            vector.tensor_scalar(junkv_sb[:, :], xb_t(ci), t, 0.0,
                                                 OP.max, OP.add, accum_out=cl).then_inc(v_sem)
                        else:
                            vector.tensor_scalar(junkv_sb[:, :], xb_t(ci), -t, 0.0,
                                                 OP.min, OP.add, accum_out=cl).then_inc(v_sem)
                    elif op[0] == 'zfh':
                        ci = op[1]
                        wait('t', ('A', ci, 2))
                        if first_acc:
                            wait('p', ('memset',))
                            first_acc = False
                        vector.scalar_tensor_tensor(
                            junkv_sb[0:96, 0:192], t1_ch(ci), 1.0,
                            psA[ci % 2][0:96, 0:192], OP.mult, OP.mult,
                            accum_out=acc_col(col_zf(ci, 0), rows=96),
                        ).then_inc(v_sem)
                    elif op[0] == 'zfd':
                        ci = op[1]
                        vector.scalar_tensor_tensor(
                            junkv_sb[0:96, 0:192], t1_ch(ci + 7), 1.0,
                            psA[ci % 2][0:96, 0:192], OP.mult, OP.mult,
                            accum_out=acc_col(col_zf(ci, 1), rows=96),
                        ).then_inc(v_sem)
                    else:  # dsnSv
                        i = op[1]
                        if i == 0:
                            wait('s', ('exp', e_ready_bank(13)))
                            vector.tensor_add(s_t('d'), e_ch(12), e_ch(13)).then_inc(v_sem)
                        else:
                            wait('s', ('exp', e_ready_bank(13 + i)))
                            vector.tensor_add(s_t('d'), s_t('d'), e_ch(13 + i)).then_inc(v_sem)

            @block.gpsimd
            def _(gpsimd):
                wait = mk_waiter(gpsimd)
                for op in pops:
                    if op[0] == 'memset':
                        gpsimd.memset(acc_sb[:, :], 0.0).then_inc(p_sem)


    return nc


# ---------------------------------------------------------------- host side --

def _interp_weights():
    s = np.linspace(np.float32(0.0), np.float32(95.0), 384).astype(np.float32)
    i0 = np.clip(np.floor(s).astype(np.int64), 0, 94)
    t = (s - i0).astype(np.float32)
    return i0, t


_CHAN_SRC = ([("preds1", c) for c in range(3)] + [("preds2", c) for c in range(2)]
             + [("preds0", c) for c in range(7)] + [("preds_dsn", c) for c in range(7)])


def _prep_core(inputs, core):
    b, half = core // 2, core % 2
    r0 = half * 192
    cy0 = 0 if half == 0 else 47
    i0, t = _interp_weights()

    uyt = np.zeros((49, 192), np.float32)
    for fy in range(192):
        f = r0 + fy
        uyt[i0[f] - cy0, fy] += np.float32(1.0) - t[f]
        uyt[i0[f] + 1 - cy0, fy] += t[f]

    ux = np.zeros((96, 384), np.float32)
    for X in range(384):
        ux[i0[X], X] += np.float32(1.0) - t[X]
        ux[i0[X] + 1, X] += t[X]
    ux = ux.astype(BF)
    uxT = np.zeros((128, 3 * 96), BF)
    for k in range(3):
        uxT[:, 96 * k: 96 * (k + 1)] = ux[:, 128 * k: 128 * (k + 1)].T

    pa = np.zeros((49, NCH * 96), BF)
    for idx, (key, ch) in enumerate(_CHAN_SRC):
        pa[:, idx * 96: (idx + 1) * 96] = inputs[key][b, ch, cy0: cy0 + 49, :].astype(BF)

    tg = np.zeros((128, 3 * 576), BF)
    for h, key in enumerate(["targets0", "targets1", "targets2"]):
        th = inputs[key][b, r0: r0 + 192, :]
        tg[:, 576 * h: 576 * (h + 1)] = (
            th.reshape(192, 3, 128).transpose(2, 1, 0).reshape(128, 576)
        ).astype(BF)

    cst = np.tile(np.asarray(BIAS_VALS + [0.0], np.float32), (128, 1))
    return {"preds": pa, "uyt": uyt.astype(BF), "ux": ux, "uxT": uxT, "tgt": tg,
            "cst": cst}


def _ncs_core(inputs, core):
    """Exact per-class pixel counts for this shard, from integer targets."""
    b, half = core // 2, core % 2
    r0 = half * 192
    ncs = []
    for ci, (head, c) in enumerate(CL):
        key = {"h1": "targets1", "h2": "targets2", "h0": "targets0"}[head]
        lab = inputs[key][b, r0: r0 + 192, :]
        ncs.append(float((lab == c).sum()))
    return ncs


def _finale(accs, ncs_all):
    lov_total = 0.0
    ce0_num = 0.0
    ced_num = 0.0
    for acc, ncs in zip(accs, ncs_all):
        cs = acc.astype(np.float64).sum(axis=0)
        head_lov = {"h1": [], "h2": [], "h0": []}
        for ci, (head, c) in enumerate(CL):
            thr = thr_of(ci)
            K = len(thr)
            base = 16 * ci
            n_c = ncs[ci]
            TF, TB = [], []
            for j, t in enumerate(thr):
                cF = cs[base + 6 + j]
                cB = cs[base + j]
                if (ci, 'F', j) in SCALAR_SET:
                    TF.append(cF)
                else:
                    TF.append(cF - N_PIX * t)
                if (ci, 'B', j) in SCALAR_SET:
                    TB.append(cB)
                else:
                    TB.append(-cB - N_PIX * t)
            TF.append(0.0)
            TB.append(0.0)
            if n_c < 0.5:
                continue
            ts_ext = list(thr) + [1.0]
            L = 0.0
            for j in range(K):
                IF = TF[j] - TF[j + 1]
                IB = TB[j] - TB[j + 1]
                d = ts_ext[j + 1] - ts_ext[j]
                L += (IF + IB) / (n_c + IB / d)
            head_lov[head].append(L)
        for head, w in (("h0", 1.0), ("h1", 0.4), ("h2", 0.4)):
            vals = head_lov[head]
            lov_total += w * (sum(vals) / max(len(vals), 1))
        ce0_num += cs[COL_LNS0] - sum(cs[16 * ci + 13] for ci in range(5, 12))
        ced_num += cs[COL_LNSD] - sum(cs[16 * ci + 14] for ci in range(5, 12))
    return ce0_num / P_GLOBAL + 0.4 * (ced_num / P_GLOBAL) + lov_total / 8.0


_NC_CACHE = None


def kernel(**inputs):
    global _NC_CACHE
    inputs = {k: np.asarray(v) for k, v in inputs.items()}
    if _NC_CACHE is None:
        _NC_CACHE = build_kernel()
    nc = _NC_CACHE
    in_maps = [_prep_core(inputs, core) for core in range(8)]
    res = run_bass_kernel_spmd(nc, in_maps, core_ids=list(range(8)))
    accs = [np.asarray(res.results[c]["acc"], dtype=np.float32) for c in range(8)]
    ncs_all = [_ncs_core(inputs, c) for c in range(8)]
    loss = _finale(accs, ncs_all)
    return np.asarray(loss, dtype=np.float32)
